# revision 19
# baseline (speedup 1.0000x reference)
"""CMRET equivariant message-passing GNN — Trainium2 Bass kernel.

One molecule per NeuronCore (the batch mask is block-diagonal: 8 molecules x
32 contiguous atoms; every pairwise term is zero across molecules), dense
32x32 local attention per molecule, no collectives.

v2 layout/schedule:
- Weights stream in ordered DMA chunks (consts+s0 first, per-layer node
  weights next) so compute starts immediately and the 3MB weight load hides
  under the first ~10us of compute.
- Edge MLPs (dk/dv silu tensors, static per layer) are emitted interleaved
  with the layer loop so ACT works on layer l+1's silus while DVE/Pool crunch
  layer l's messages.
- The cosine cutoff is folded into dv1/dv2/dv3 once per layer (they are
  static), so messages use Xp = exp(logits) directly; no Ec/lnco machinery.
- All node-matmul biases are accumulated in PSUM via bias-row x ones-row
  matmuls; downstream engines read PSUM directly (no ACT bias passes).
- Per-edge products run on DVE in bf16 (2x mode); segmented b-reduces are
  split between DVE and the otherwise idle Pool (gpsimd) engine.
"""

import numpy as np

RC = 5.0
N_ATOM = 256
N_MOL = 8
NA = 32          # atoms per molecule
F = 128
K = 50
L = 4
H = 4
Dh = 32
TEMP = 2.0
NE = NA * NA     # dense per-molecule edges (diag masked)
GAMMA = 0.5 / (RC / (K - 1)) ** 2
TEMPERATURE = TEMP * np.sqrt(Dh)
PI = float(np.pi)


def _chunk_layouts():
    """DMA chunks, each a packed [p, w] block list: name -> (col0, parts, w)."""
    chunks = {}

    def lay(entries):
        offs, c = {}, 0
        for n, p, w in entries:
            offs[n] = (c, p, w)
            c += w
        return offs, c

    # consts + per-molecule state; first to arrive
    chunks["WallA"] = lay([
        ("s0T", F, NA), ("R", NA, 3),
        ("halfdmask", NA, NA), ("diagI", NA, NA), ("mub", K, 1),
        ("ones128inv", F, 1), ("ones1", 1, F), ("ones32", 1, NA),
    ])
    # bf16 fast-path PE operands
    enth = [("ones1h", 1, F), ("ones32h", 1, NA), ("HH", F, F)]
    for l in range(L):
        enth += [(f"Wdk{l}", K, F), (f"Wdv{l}", K, 3 * F)]
    chunks["WallH"] = lay(enth)
    # per-layer f32 node weights (folded LN affine + temperature)
    for l in range(L):
        chunks[f"W{l}"] = lay([
            (f"Wq{l}", F, F), (f"bqr{l}", 1, F),
            (f"Wk{l}", F, F), (f"bkr{l}", 1, F),
            (f"Wv{l}", F, 3 * F), (f"bvr{l}", 1, 3 * F),
            (f"Wo{l}", F, 3 * F), (f"bor{l}", 1, 3 * F),
            (f"U1{l}", F, F), (f"U2{l}", F, F), (f"U3{l}", F, F),
            (f"bdk{l}", F, 1), (f"bdv{l}", F, 3),
        ])
    chunks["WF"] = lay([("w1p", F, F // 2), ("b1r", 1, F // 2), ("w2", F // 2, 1)])
    return chunks


def _host_prep(inp):
    """Fold LN affine + temperature into weights; pack into chunked walls."""
    f32 = np.float32
    import ml_dtypes
    bf16 = ml_dtypes.bfloat16
    Z = np.asarray(inp["Z"]).reshape(-1)
    Rfull = np.asarray(inp["R"], f32).reshape(N_ATOM, 3)
    embed = np.asarray(inp["embed"], f32)
    s0 = embed[Z]                                   # (256, F) gather on host

    vals = {}
    for l in range(L):
        g = np.asarray(inp["ln_g"][l], f32)
        b = np.asarray(inp["ln_b"][l], f32)
        Wq = np.asarray(inp["Wq"][l], f32)
        Wk = np.asarray(inp["Wk"][l], f32)
        Wv = np.asarray(inp["Wv"][l], f32)
        vals[f"Wq{l}"] = g[:, None] * Wq / TEMPERATURE
        vals[f"bqr{l}"] = (b @ Wq / TEMPERATURE).reshape(1, F)
        vals[f"Wk{l}"] = g[:, None] * Wk
        vals[f"bkr{l}"] = (b @ Wk).reshape(1, F)
        vals[f"Wv{l}"] = g[:, None] * Wv
        vals[f"bvr{l}"] = (b @ Wv).reshape(1, 3 * F)
        vals[f"Wdk{l}"] = np.asarray(inp["Wdk"][l], f32)
        vals[f"bdk{l}"] = np.asarray(inp["bdk"][l], f32).reshape(F, 1)
        vals[f"Wdv{l}"] = np.asarray(inp["Wdv"][l], f32)
        vals[f"bdv{l}"] = np.asarray(inp["bdv"][l], f32).reshape(3, F).T
        vals[f"Wo{l}"] = np.asarray(inp["Wo"][l], f32)
        vals[f"bor{l}"] = np.asarray(inp["bo"][l], f32).reshape(1, 3 * F)
        vals[f"U1{l}"] = np.asarray(inp["U1"][l], f32)
        vals[f"U2{l}"] = np.asarray(inp["U2"][l], f32)
        vals[f"U3{l}"] = np.asarray(inp["U3"][l], f32)

    lg = np.asarray(inp["lnf_g"], f32)
    lb = np.asarray(inp["lnf_b"], f32)
    w1 = np.asarray(inp["out_w1"], f32)
    vals["w1p"] = lg[:, None] * w1
    vals["b1r"] = (lb @ w1 + np.asarray(inp["out_b1"], f32)).reshape(1, F // 2)
    vals["w2"] = np.asarray(inp["out_w2"], f32).reshape(F // 2, 1)

    hh = np.zeros((F, F), f32)
    for h in range(H):
        hh[h * Dh:(h + 1) * Dh, h * Dh:(h + 1) * Dh] = 1.0
    vals["HH"] = hh
    eye = np.eye(NA, dtype=f32)
    vals["halfdmask"] = (0.5 * (1.0 - eye)).astype(f32)
    vals["diagI"] = eye
    mu = np.linspace(0.0, RC, K).astype(f32)
    vals["mub"] = (-np.sqrt(GAMMA) * mu).reshape(K, 1).astype(f32)
    vals["ones128inv"] = np.full((F, 1), 1.0 / F, f32)
    vals["ones1"] = np.ones((1, F), f32)
    vals["ones1h"] = np.ones((1, F), f32)
    vals["ones32"] = np.ones((1, NA), f32)
    vals["ones32h"] = np.ones((1, NA), f32)

    chunks = _chunk_layouts()
    arrays = {}
    for cname, (offs, C) in chunks.items():
        dt = f32 if cname in ("WallA", "WF") else bf16
        arr = np.zeros((F, C), dtype=dt)
        for n, (c0, p, w) in offs.items():
            if cname == "WallA" and n in ("s0T", "R"):
                continue  # per-molecule, filled below
            arr[0:p, c0:c0 + w] = vals[n].astype(dt)
        arrays[cname] = np.ascontiguousarray(arr)

    offsA, CA = chunks["WallA"]
    wallAs = []
    for m in range(N_MOL):
        wl = arrays["WallA"].copy()
        c0, p, w = offsA["s0T"]
        wl[0:p, c0:c0 + w] = s0[m * NA:(m + 1) * NA].T
        c0, p, w = offsA["R"]
        wl[0:p, c0:c0 + w] = Rfull[m * NA:(m + 1) * NA]
        wallAs.append(np.ascontiguousarray(wl))
    b2 = float(np.asarray(inp["out_b2"]).reshape(-1)[0])
    return arrays, wallAs, b2


_CACHE = {}


def kernel(**inputs):
    from concourse import bass_utils

    arrays, wallAs, b2 = _host_prep(inputs)

    key = ("nc", b2)
    if key not in _CACHE:
        _CACHE[key] = _build(b2)
    nc = _CACHE[key]

    in_maps = []
    for m in range(N_MOL):
        d = {cn: arrays[cn] for cn in arrays if cn != "WallA"}
        d["WallA"] = wallAs[m]
        in_maps.append(d)
    res = bass_utils.run_bass_kernel_spmd(nc, in_maps, core_ids=list(range(N_MOL)))
    out = np.concatenate([r["energy"].reshape(1) for r in res.results]).reshape(N_MOL, 1)
    return out.astype(np.float32)


def _patch_tile_drain():
    """The Tile kernel-tail drain carries one sem-wait per active processor;
    this walrus build caps sync waits per CTRL instruction. Split the waits
    onto individual SP nops (same semantics: all run before the exit
    barrier on the sync engine)."""
    import concourse.tile as tile_mod
    import bass_rust
    from concourse.vector_clock import ScopedClock

    if getattr(tile_mod.TileContext, "_drain_split_patched", False):
        return

    def _drain_and_barrier(self, tick_clock, wait_clock):
        nc = self.nc
        drain_inst = nc.sync.drain()
        wait_clock.add_sem_waits(
            drain_inst.ins, ScopedClock({None: tick_clock.global_clock})
        )
        si = drain_inst.ins.sync_info
        waits = list(si.on_wait or []) if si is not None else []
        if len(waits) > 1:
            drain_inst.ins.sync_info = bass_rust.SyncInfo(
                on_wait=waits[:1], on_update=list(si.on_update or []))
            for w in waits[1:]:
                nop = nc.sync.nop(nofuse=True)
                nop.ins.sync_info = bass_rust.SyncInfo(on_wait=[w], on_update=[])
        nc.all_engine_barrier()
        popped = nc._tile_sem_poison_stack.pop()
        assert popped is self._sem_poison
        nc.clear_and_free_semaphores(list(self.sems.allocated().values()))
        nc.all_engine_barrier()

    tile_mod.TileContext._drain_and_barrier = _drain_and_barrier
    tile_mod.TileContext._drain_split_patched = True


def _split_sync_waits(nc, mybir):
    """This walrus build rejects instructions carrying more than one sync
    wait ("Too many sync wait commands"). Hoist extra waits onto inserted
    same-engine NoOps immediately before the instruction — the engine
    sequencer blocks on each in turn, preserving the happens-before."""
    import bass_rust

    n_split = 0
    for fn in nc.m.functions:
        for bb in fn.blocks:
            changed = False
            new = []
            for ins in bb.instructions:
                si = ins.sync_info
                waits = list(si.on_wait or []) if si is not None else []
                if len(waits) > 1:
                    for i, w in enumerate(waits[:-1]):
                        nop = mybir.InstNoOp(name=f"{ins.name}-sw{i}")
                        nop.engine = ins.engine
                        nop.sync_info = bass_rust.SyncInfo(on_wait=[w], on_update=[])
                        nc.inst_map[nop.name] = nop
                        new.append(nop)
                    ins.sync_info = bass_rust.SyncInfo(
                        on_wait=[waits[-1]], on_update=list(si.on_update or []))
                    changed = True
                    n_split += 1
                new.append(ins)
            if changed:
                bb.instructions = new
    return n_split


def _build(b2):
    import concourse.bass as bass
    import concourse.mybir as mybir
    import concourse.tile as tile

    _patch_tile_drain()

    f32 = mybir.dt.float32
    bf16 = mybir.dt.bfloat16
    AF = mybir.ActivationFunctionType
    ALU = mybir.AluOpType
    AX = mybir.AxisListType

    def bcast_inner(ap, outer, inner):
        # (P, n) -> (P, outer(step), inner(bcast)): value[p, i, j] = ap[p, i]
        return bass.AP(tensor=ap.tensor, offset=ap.offset,
                       ap=[ap.ap[0], [ap.ap[1][0], outer], [0, inner]])

    def bcast_outer(ap, outer, inner):
        # (P, n) -> (P, outer(bcast), inner(step)): value[p, i, j] = ap[p, j]
        return bass.AP(tensor=ap.tensor, offset=ap.offset,
                       ap=[ap.ap[0], [0, outer], [ap.ap[1][0], inner]])

    nc = bass.Bass()
    chunks = _chunk_layouts()
    dram = {}
    for cn, (offs, C) in chunks.items():
        dt = f32 if cn in ("WallA", "WF") else bf16
        dram[cn] = nc.dram_tensor(cn, [F, C], dt, kind="ExternalInput")
    energy = nc.dram_tensor("energy", [1, 1], f32, kind="ExternalOutput")

    with tile.TileContext(nc) as tc:
        with tc.tile_pool(name="const", bufs=1) as cp, \
             tc.tile_pool(name="geo", bufs=1) as gp, \
             tc.tile_pool(name="small", bufs=3) as sp, \
             tc.tile_pool(name="wide", bufs=10) as wp, \
             tc.tile_pool(name="psE", bufs=2, space="PSUM") as psE, \
             tc.tile_pool(name="psX", bufs=1, space="PSUM") as psX, \
             tc.tile_pool(name="psS", bufs=2, space="PSUM") as psS, \
             tc.tile_pool(name="psN", bufs=2, space="PSUM") as psN:

            # ---- chunked weight DMAs, in priority order ----
            walls = {}
            W = {}

            def load_chunk(cn):
                offs, C = chunks[cn]
                dt = f32 if cn in ("WallA", "WF") else bf16
                t = cp.tile([F, C], dt, tag=cn, name=cn)
                nc.sync.dma_start(out=t[:], in_=dram[cn][:])
                walls[cn] = t
                for n, (c0, p, w) in offs.items():
                    W[n] = t[0:p, c0:c0 + w]

            order = ["WallA", "W0", "WallH"]
            for cn in order:
                offs, C = chunks[cn]
                dt = f32 if cn in ("WallA", "WF") else bf16
                t = cp.tile([F, C], dt, tag=cn, name=cn)
                nc.sync.dma_start(out=t[:], in_=dram[cn][:])
                walls[cn] = t
                for n, (c0, p, w) in offs.items():
                    W[n] = t[0:p, c0:c0 + w]
                if cn == "WallA":
                    # geometry input: R replicated across partitions; must be
                    # early on the serial DMA-engines resource
                    Rb = gp.tile([NA, NA * 3], f32, tag="Rb")
                    rc0 = offs["R"][0]
                    CA = C
                    nc.sync.dma_start(
                        out=Rb[:], in_=bass.AP(tensor=dram[cn][:].tensor, offset=rc0,
                                               ap=[[0, NA], [CA, NA], [1, 3]]))

            # small constant bias tiles for ACT (only 0.0/1.0 have const APs)
            b30 = cp.tile([NA, 1], f32, tag="b30", name="b30")
            nc.vector.memset(b30[:], 1e-30)
            bpi2 = cp.tile([NA, 1], f32, tag="bpi2", name="bpi2")
            nc.vector.memset(bpi2[:], PI / 2)
            beps = cp.tile([1, 1], f32, tag="beps", name="beps")
            nc.vector.memset(beps[:], 1e-5)

            # persistent state (sT first: LN(s0) is on the spine immediately)
            sT = gp.tile([F, NA], f32, tag="sT")
            nc.vector.tensor_copy(sT[:], W["s0T"][:])
            oT = gp.tile([F, NA], f32, tag="oT")
            nc.vector.memset(oT[:], 0.0)
            vT = []
            for c in range(3):
                t = gp.tile([F, NA], f32, tag=f"vT{c}")
                nc.gpsimd.memset(t[:], 0.0)
                vT.append(t)

            # =========== geometry ===========
            V = gp.tile([NA, NA, 3], f32, tag="V")      # vec[a, b, c] = R[a,c] - R[b,c]
            Ra = W["R"][:]
            Ra_b = bass.AP(tensor=Ra.tensor, offset=Ra.offset,
                           ap=[Ra.ap[0], [0, NA], [Ra.ap[1][0], 3]])
            nc.vector.tensor_sub(V[:], Ra_b, Rb[:].rearrange("p (b c) -> p b c", c=3))
            V2 = sp.tile([NA, NA, 3], f32, tag="V2")
            nc.vector.tensor_mul(V2[:], V[:], V[:])
            d2 = sp.tile([NA, NA], f32, tag="d2")
            nc.vector.reduce_sum(d2[:], V2[:], axis=AX.X)
            lnd2 = sp.tile([NA, NA], f32, tag="lnd2")
            nc.scalar.activation(lnd2[:], d2[:], AF.Ln, bias=b30[:])
            dmat = gp.tile([NA, NA], f32, tag="dmat")   # d = exp(0.5*ln(d2))
            nc.scalar.activation(dmat[:], lnd2[:], AF.Exp, scale=0.5)
            dsafe = sp.tile([NA, NA], f32, tag="dsafe")
            nc.vector.tensor_add(dsafe[:], dmat[:], W["diagI"][:])
            invd = sp.tile([NA, NA], f32, tag="invd")
            nc.vector.reciprocal(invd[:], dsafe[:])
            vn = gp.tile([NA, NA, 3], f32, tag="vn")    # vec_norm (diag exactly 0)
            iap = invd[:]
            nc.vector.tensor_mul(vn[:], V[:], bass.AP(tensor=iap.tensor, offset=iap.offset,
                                                      ap=[iap.ap[0], [iap.ap[1][0], NA], [0, 3]]))
            vn_b = gp.tile([NA, NA, 3], bf16, tag="vn_b", name="vn_b")
            nc.vector.tensor_copy(vn_b[:], vn[:])

            # cutoff: co = 0.5*(cos(pi*d/RC)+1)*(d<=RC), diag zero
            stepm = sp.tile([NA, NA], f32, tag="stepm")
            nc.vector.tensor_scalar(out=stepm[:], in0=dmat[:], scalar1=RC, scalar2=None,
                                    op0=ALU.is_le)
            d_cl = sp.tile([NA, NA], f32, tag="d_cl")
            nc.vector.tensor_scalar(out=d_cl[:], in0=dmat[:], scalar1=RC, scalar2=None,
                                    op0=ALU.min)
            s1 = sp.tile([NA, NA], f32, tag="s1")
            nc.scalar.activation(s1[:], d_cl[:], AF.Sin, bias=bpi2[:], scale=-PI / RC)
            m32 = sp.tile([NA, NA], f32, tag="m32")
            nc.vector.tensor_mul(m32[:], stepm[:], W["halfdmask"][:])
            co_a = gp.tile([NA, NA], f32, tag="co_a")
            nc.vector.scalar_tensor_tensor(co_a[:], s1[:], 1.0, m32[:],
                                           op0=ALU.add, op1=ALU.mult)
            co_b = gp.tile([NA, NA], bf16, tag="co_b", name="co_b")
            nc.vector.tensor_copy(co_b[:], co_a[:])

            # row-ized [1, NE] copies (spread across DMA queues)
            dE = gp.tile([1, NE], f32, tag="dE", name="dE")
            nc.sync.dma_start(out=dE[:], in_=dmat[:])
            coE = gp.tile([1, NE], bf16, tag="coE", name="coE")
            nc.sync.dma_start(out=coE[:], in_=co_b[:])
            vnrow = [gp.tile([1, NE], bf16, tag=f"vnrow{c}", name=f"vnrow{c}")
                     for c in range(3)]
            nc.sync.dma_start(out=vnrow[0][:], in_=vn_b[:, :, 0])
            nc.sync.dma_start(out=vnrow[1][:], in_=vn_b[:, :, 1])
            nc.sync.dma_start(out=vnrow[2][:], in_=vn_b[:, :, 2])
            # late weight chunks: gate each behind the last rowize DMA so they
            # cannot grab the serial DMA engines before the geometry rows land
            for cn in ["W1", "W2", "W3", "WF"]:
                offs, C = chunks[cn]
                dt = f32 if cn in ("WallA", "WF") else bf16
                t = cp.tile([F, C], dt, tag=cn, name=cn)
                nc.vector.tensor_copy(t[0:1, 0:1], vnrow[2][0:1, 0:1])
                nc.sync.dma_start(out=t[:], in_=dram[cn][:])
                walls[cn] = t
                for n, (c0, p, w) in offs.items():
                    W[n] = t[0:p, c0:c0 + w]

            # RBF edge features erbf[K, NE] = exp(-gamma (d - mu)^2), bf16
            erbf = gp.tile([K, NE], bf16, tag="erbf", name="erbf")
            for hh_ in range(2):
                sl = slice(hh_ * 512, (hh_ + 1) * 512)
                pb = psE.tile([K, 512], f32, tag="edge", name="pb_rbf")
                nc.tensor.matmul(pb[:], W["ones1"][0:1, 0:K], dE[:, sl],
                                 start=True, stop=True)
                sq = sp.tile([K, 512], f32, tag="rbf_sq")
                nc.scalar.activation(sq[:], pb[:], AF.Square, bias=W["mub"][:],
                                     scale=float(np.sqrt(GAMMA)))
                nc.scalar.activation(erbf[:, sl], sq[:], AF.Exp, scale=-1.0)

            # broadcast co50 [K,NE] now (feeds e_full); co128/vnE are deferred
            # until after layer-0's edge MLP so they don't block its silus
            co50 = gp.tile([K, NE], bf16, tag="co50")
            for hh_ in range(2):
                sl = slice(hh_ * 512, (hh_ + 1) * 512)
                pb = psE.tile([K, 512], f32, tag="edge", name="pb_co50")
                nc.tensor.matmul(pb[:], W["ones1h"][0:1, 0:K], coE[:, sl],
                                 start=True, stop=True)
                nc.scalar.copy(co50[:, sl], pb[:])

            def bcast_co_vne():
                co128 = gp.tile([F, NE], bf16, tag="co128")
                for hh_ in range(2):
                    sl = slice(hh_ * 512, (hh_ + 1) * 512)
                    pb2 = psE.tile([F, 512], f32, tag="edge", name="pb_co128")
                    nc.tensor.matmul(pb2[:], W["ones1h"][:], coE[:, sl],
                                     start=True, stop=True)
                    nc.scalar.copy(co128[:, sl], pb2[:])
                vnE = []
                for c in range(3):
                    t = gp.tile([F, NE], bf16, tag=f"vnE{c}", name=f"vnE{c}")
                    for hh_ in range(2):
                        sl = slice(hh_ * 512, (hh_ + 1) * 512)
                        pb = psE.tile([F, 512], f32, tag="edge", name="pb_vne")
                        nc.tensor.matmul(pb[:], W["ones1h"][:], vnrow[c][:, sl],
                                         start=True, stop=True)
                        if c == 2:
                            nc.scalar.copy(t[:, sl], pb[:])
                        else:
                            nc.vector.tensor_copy(t[:, sl], pb[:])
                    vnE.append(t)
                return co128, vnE

            e_full = gp.tile([K, NE], bf16, tag="e_full", name="e_full")
            nc.vector.tensor_mul(e_full[:], erbf[:], co50[:])

            def e3(t):
                return t[:].rearrange("p (a b) -> p a b", a=NA)

            # edge MLP for one layer: dk + dv1/2/3 with co folded into dv
            def edge_mlp(l):
                dk = gp.tile([F, NE], bf16, tag=f"dk{l}")
                for h in range(2):
                    sl = slice(h * 512, (h + 1) * 512)
                    pm = psE.tile([F, 512], f32, tag="edge")
                    nc.tensor.matmul(pm[:], W[f"Wdk{l}"], e_full[:, sl],
                                     start=True, stop=True)
                    nc.scalar.activation(dk[:, sl], pm[:], AF.Silu, bias=W[f"bdk{l}"][:])
                dvl = []
                for c in range(3):
                    dv = gp.tile([F, NE], bf16, tag=f"dv{l}_{c}")
                    for h in range(2):
                        sl = slice(h * 512, (h + 1) * 512)
                        pm = psE.tile([F, 512], f32, tag="edge")
                        nc.tensor.matmul(pm[:], W[f"Wdv{l}"][:, c * F:(c + 1) * F],
                                         e_full[:, sl], start=True, stop=True)
                        nc.scalar.activation(dv[:, sl], pm[:], AF.Silu,
                                             bias=W[f"bdv{l}"][:, c:c + 1])
                    dvl.append(dv)
                return dk, dvl

            def layernorm_f(inT):
                # LN stats over the feature (partition) axis via PE ones-matmuls
                sq = sp.tile([F, NA], f32, tag="lnsq")
                nc.scalar.activation(sq[:], inT[:], AF.Square)
                stat = psN.tile([1, 2 * NA], f32, tag="nst")
                nc.tensor.matmul(stat[:, 0:NA], W["ones128inv"][:], inT[:],
                                 start=True, stop=True)
                nc.tensor.matmul(stat[:, NA:2 * NA], W["ones128inv"][:], sq[:],
                                 start=True, stop=True)
                statm = stat[:, 0:NA]
                varr = sp.tile([1, NA], f32, tag="varr")
                nc.scalar.activation(varr[:], statm, AF.Square)
                nc.vector.tensor_sub(varr[:], stat[:, NA:2 * NA], varr[:])
                rb = sp.tile([1, 2 * NA], f32, tag="rb")
                lnv = sp.tile([1, NA], f32, tag="lnv")
                nc.scalar.activation(lnv[:], varr[:], AF.Ln, bias=beps[:])
                nc.scalar.activation(rb[:, 0:NA], lnv[:], AF.Exp, scale=-0.5)   # rstd
                nc.vector.tensor_mul(rb[:, NA:2 * NA], statm, rb[:, 0:NA])      # mu*rstd
                bc = psN.tile([F, 2 * NA], f32, tag="nst")
                nc.tensor.matmul(bc[:], W["ones1"][:], rb[:], start=True, stop=True)
                xh = sp.tile([F, NA], f32, tag="xhatT")
                nc.vector.tensor_mul(xh[:], inT[:], bc[:, 0:NA])
                nc.vector.tensor_sub(xh[:], xh[:], bc[:, NA:2 * NA])
                return xh

            # layer-0 edge MLP emitted before the loop so ACT starts early
            dks, dvs = [None] * L, [None] * L
            dks[0], dvs[0] = edge_mlp(0)
            co128, vnE = bcast_co_vne()

            # =========== interaction layers ===========
            for l in range(L):
                first = l == 0

                # LN first: its PE stats must not queue behind the U-matmuls
                # (which wait on late vT updates from the previous layer)
                xhatT = layernorm_f(sT)
                xhb = sp.tile([F, NA], bf16, tag="xhb")
                nc.vector.tensor_copy(xhb[:], xhatT[:])

                # U-matmuls + dot chain: need only prev-layer vT; overlap LN
                ou = psS.tile([F, 512], f32, tag="qkv")
                if not first:
                    vTb = []
                    for c in range(3):
                        vb = sp.tile([F, NA], bf16, tag=f"vTb{c}")
                        nc.gpsimd.tensor_copy(vb[:], vT[c][:])
                        vTb.append(vb)
                    for c in range(3):
                        nc.tensor.matmul(ou[:, (3 + c) * NA:(4 + c) * NA], W[f"U1{l}"][:],
                                         vTb[c][:], start=True, stop=True)
                        nc.tensor.matmul(ou[:, (6 + c) * NA:(7 + c) * NA], W[f"U2{l}"][:],
                                         vTb[c][:], start=True, stop=True)
                        nc.tensor.matmul(ou[:, (9 + c) * NA:(10 + c) * NA], W[f"U3{l}"][:],
                                         vTb[c][:], start=True, stop=True)
                    us = sp.tile([F, 9 * NA], f32, tag="us")
                    nc.vector.tensor_copy(us[:], ou[:, 3 * NA:12 * NA])
                    dot = sp.tile([F, NA], f32, tag="dot")
                    pc = sp.tile([F, NA], f32, tag="dotp")
                    nc.gpsimd.tensor_mul(dot[:], us[:, 0:NA], us[:, 3 * NA:4 * NA])
                    for c in range(1, 3):
                        nc.gpsimd.tensor_mul(pc[:], us[:, c * NA:(c + 1) * NA],
                                             us[:, (3 + c) * NA:(4 + c) * NA])
                        nc.gpsimd.tensor_add(dot[:], dot[:], pc[:])

                # node matmuls with PE-accumulated biases; all stay in PSUM
                qkv = psS.tile([F, 512], f32, tag="qkv")

                def node_mm(dst, wap, brow):
                    nc.tensor.matmul(dst, brow, W["ones32h"][:], start=True, stop=False)
                    nc.tensor.matmul(dst, wap, xhb[:], start=False, stop=True)

                node_mm(qkv[:, 0:NA], W[f"Wq{l}"][:], W[f"bqr{l}"][:])
                node_mm(qkv[:, NA:2 * NA], W[f"Wk{l}"][:], W[f"bkr{l}"][:])
                for c in range(3):
                    node_mm(qkv[:, (2 + c) * NA:(3 + c) * NA],
                            W[f"Wv{l}"][:, c * F:(c + 1) * F],
                            W[f"bvr{l}"][:, c * F:(c + 1) * F])
                q_ap = qkv[:, 0:NA]
                kb = sp.tile([F, NA], bf16, tag="kb")
                nc.scalar.copy(kb[:], qkv[:, NA:2 * NA])
                k_ap = kb[:]
                val = [qkv[:, (2 + c) * NA:(3 + c) * NA] for c in range(3)]
                # bf16 copies + G on Pool (off the DVE spine)
                val1b = sp.tile([F, NA], bf16, tag="val1b")
                nc.scalar.copy(val1b[:], val[0])
                val3b = sp.tile([F, NA], bf16, tag="val3b")
                nc.scalar.copy(val3b[:], val[2])
                if not first:
                    val2b = sp.tile([F, NA], bf16, tag="val2b")
                    nc.scalar.copy(val2b[:], val[1])
                    G = []
                    for c in range(3):
                        g = sp.tile([F, NA], bf16, tag=f"G{c}")
                        nc.gpsimd.tensor_mul(g[:], val2b[:], vT[c][:])
                        G.append(g)

                # logits: qk = q (x) k; prod = qk * dk; head-sum via HH; exp
                qk = wp.tile([F, NA, NA], bf16, tag="w")
                prod = wp.tile([F, NA, NA], bf16, tag="w")
                Xp = wp.tile([F, NE], bf16, tag="w")
                lps = psX.tile([F, NE], f32, tag="lg")
                for hch in range(2):
                    asl = slice(hch * 16, (hch + 1) * 16)      # a-halves
                    csl = slice(hch * 512, (hch + 1) * 512)
                    nc.vector.tensor_mul(qk[:, asl, :], bcast_inner(q_ap[:, asl], 16, NA),
                                         bcast_outer(k_ap, 16, NA))
                    nc.vector.tensor_mul(prod[:, asl, :], qk[:, asl, :],
                                         e3(dks[l])[:, asl, :])
                    nc.tensor.matmul(lps[:, csl],
                                     W["HH"], prod[:].rearrange("p a b -> p (a b)")[:, csl],
                                     start=True, stop=True)
                    nc.scalar.activation(Xp[:, csl], lps[:, csl], AF.Exp)

                # W1 = dv1 * val1b rides in the DVE gap while PE/ACT do exp
                W1c = wp.tile([F, NA, NA], bf16, tag="w")
                nc.vector.tensor_mul(W1c[:], e3(dvs[l][0]), bcast_outer(val1b[:], NA, NA))

                # edge MLP for the next layer rides behind this layer's ACT work
                if l + 1 < L:
                    dks[l + 1], dvs[l + 1] = edge_mlp(l + 1)

                # spine: S -> invD -> Y -> P1v -> ds -> Wo -> dx -> sT
                S = sp.tile([F, NA], f32, tag="S")
                nc.vector.reduce_sum(S[:, 0:16], e3(Xp)[:, 0:16, :], axis=AX.X)
                nc.vector.reduce_sum(S[:, 16:NA], e3(Xp)[:, 16:NA, :], axis=AX.X)
                xap = Xp[:]
                diag_ap = bass.AP(tensor=xap.tensor, offset=xap.offset,
                                  ap=[xap.ap[0], [(NA + 1) * xap.ap[1][0], NA]])
                invD = sp.tile([F, NA], f32, tag="invD")
                nc.vector.tensor_sub(invD[:], S[:], diag_ap)
                nc.vector.reciprocal(invD[:], invD[:])
                Y = wp.tile([F, NA, NA], bf16, tag="w")
                P1v = wp.tile([F, NA, NA], bf16, tag="w")
                dsT = sp.tile([F, NA], f32, tag="dsT")
                for hch in range(2):
                    asl = slice(hch * 16, (hch + 1) * 16)
                    nc.vector.tensor_mul(Y[:, asl, :], e3(Xp)[:, asl, :],
                                         e3(co128)[:, asl, :])
                    nc.vector.tensor_mul(P1v[:, asl, :], Y[:, asl, :], W1c[:, asl, :])
                    nc.vector.reduce_sum(dsT[:, asl], P1v[:, asl, :], axis=AX.X)
                dsb = sp.tile([F, NA], bf16, tag="dsb")
                nc.vector.tensor_mul(dsb[:], dsT[:], invD[:])

                def node_mm2(dst, wap, brow, mov):
                    nc.tensor.matmul(dst, brow, W["ones32h"][:], start=True, stop=False)
                    nc.tensor.matmul(dst, wap, mov, start=False, stop=True)

                for c in range(3):
                    node_mm2(ou[:, c * NA:(c + 1) * NA], W[f"Wo{l}"][:, c * F:(c + 1) * F],
                             W[f"bor{l}"][:, c * F:(c + 1) * F], dsb[:])
                o1, o2, o3 = (ou[:, c * NA:(c + 1) * NA] for c in range(3))
                dx = sp.tile([F, NA], f32, tag="dx")
                if first:
                    nc.vector.tensor_copy(dx[:], o2)
                else:
                    nc.vector.tensor_mul(dx[:], o3, dot[:])
                    nc.vector.tensor_add(dx[:], dx[:], o2)
                nc.vector.tensor_add(sT[:], sT[:], dx[:])
                nc.vector.tensor_add(oT[:], oT[:], dx[:])

                # dw messages trail the spine; only next layer's U/G need them
                X3 = wp.tile([F, NA, NA], bf16, tag="w")
                nc.vector.tensor_mul(X3[:], Y[:], bcast_outer(val3b[:], NA, NA))
                P3 = wp.tile([F, NA, NA], bf16, tag="w")
                nc.vector.tensor_mul(P3[:], X3[:], e3(dvs[l][2]))
                if not first:
                    P2 = wp.tile([F, NA, NA], bf16, tag="w")
                    nc.vector.tensor_mul(P2[:], Y[:], e3(dvs[l][1]))
                dwm = sp.tile([F, 3, NA], f32, tag="dwm")
                for c in range(3):
                    tt = wp.tile([F, NA, NA], bf16, tag="w")
                    if c == 2:
                        nc.gpsimd.tensor_mul(tt[:], P3[:], e3(vnE[c]))
                    else:
                        nc.vector.tensor_mul(tt[:], P3[:], e3(vnE[c]))
                    if not first:
                        rr = wp.tile([F, NA, NA], bf16, tag="w")
                        if c == 2:
                            nc.gpsimd.tensor_mul(rr[:], P2[:], bcast_outer(G[c][:], NA, NA))
                            nc.vector.tensor_add(tt[:], tt[:], rr[:])
                        else:
                            nc.vector.tensor_mul(rr[:], P2[:], bcast_outer(G[c][:], NA, NA))
                            nc.vector.tensor_add(tt[:], tt[:], rr[:])
                    nc.vector.reduce_sum(dwm[:, c, :], tt[:], axis=AX.X)
                iap2 = invD[:]
                nc.vector.tensor_mul(dwm[:], dwm[:],
                                     bass.AP(tensor=iap2.tensor, offset=iap2.offset,
                                             ap=[iap2.ap[0], [0, 3], [iap2.ap[1][0], NA]]))
                dwT = [dwm[:, c, :] for c in range(3)]
                if first:
                    for c in range(3):
                        nc.gpsimd.tensor_copy(vT[c][:], dwT[c])
                else:
                    o1s = sp.tile([F, NA], f32, tag="o1s")
                    nc.vector.tensor_copy(o1s[:], o1)
                    for c in range(3):
                        t3 = sp.tile([F, NA], f32, tag="t3")
                        nc.gpsimd.tensor_mul(t3[:], o1s[:], us[:, (6 + c) * NA:(7 + c) * NA])
                        nc.gpsimd.tensor_add(vT[c][:], vT[c][:], dwT[c])
                        nc.gpsimd.tensor_add(vT[c][:], vT[c][:], t3[:])

            # =========== final LN + output MLP ===========
            xo = layernorm_f(oT)
            y_p = psS.tile([F // 2, NA], f32, tag="qkv")
            nc.tensor.matmul(y_p[:], W["b1r"][:], W["ones32"][:], start=True, stop=False)
            nc.tensor.matmul(y_p[:], W["w1p"][:], xo[:], start=False, stop=True)
            a1 = sp.tile([F // 2, NA], f32, tag="a1")
            nc.scalar.activation(a1[:], y_p[:], AF.Silu)
            asum = sp.tile([F // 2, 1], f32, tag="asum")
            nc.vector.reduce_sum(asum[:], a1[:], axis=AX.X)
            en_p = psS.tile([1, 1], f32, tag="qkv")
            nc.tensor.matmul(en_p[:], W["w2"][:], asum[:], start=True, stop=True)
            en = sp.tile([1, 1], f32, tag="en")
            nc.vector.tensor_scalar(out=en[:], in0=en_p[:], scalar1=float(NA * b2),
                                    scalar2=None, op0=ALU.add)
            nc.sync.dma_start(out=energy[:], in_=en[:])

    _split_sync_waits(nc, mybir)
    nc.finalize()
    return nc


# revision 20
# speedup vs baseline: 1.0289x; 1.0289x over previous
"""CMRET equivariant message-passing GNN — Trainium2 Bass kernel.

One molecule per NeuronCore (the batch mask is block-diagonal: 8 molecules x
32 contiguous atoms; every pairwise term is zero across molecules), dense
32x32 local attention per molecule, no collectives.

v2 layout/schedule:
- Weights stream in ordered DMA chunks (consts+s0 first, per-layer node
  weights next) so compute starts immediately and the 3MB weight load hides
  under the first ~10us of compute.
- Edge MLPs (dk/dv silu tensors, static per layer) are emitted interleaved
  with the layer loop so ACT works on layer l+1's silus while DVE/Pool crunch
  layer l's messages.
- The cosine cutoff is folded into dv1/dv2/dv3 once per layer (they are
  static), so messages use Xp = exp(logits) directly; no Ec/lnco machinery.
- All node-matmul biases are accumulated in PSUM via bias-row x ones-row
  matmuls; downstream engines read PSUM directly (no ACT bias passes).
- Per-edge products run on DVE in bf16 (2x mode); segmented b-reduces are
  split between DVE and the otherwise idle Pool (gpsimd) engine.
"""

import numpy as np

RC = 5.0
N_ATOM = 256
N_MOL = 8
NA = 32          # atoms per molecule
F = 128
K = 50
L = 4
H = 4
Dh = 32
TEMP = 2.0
NE = NA * NA     # dense per-molecule edges (diag masked)
GAMMA = 0.5 / (RC / (K - 1)) ** 2
TEMPERATURE = TEMP * np.sqrt(Dh)
PI = float(np.pi)


def _chunk_layouts():
    """DMA chunks, each a packed [p, w] block list: name -> (col0, parts, w)."""
    chunks = {}

    def lay(entries):
        offs, c = {}, 0
        for n, p, w in entries:
            offs[n] = (c, p, w)
            c += w
        return offs, c

    # consts + per-molecule state; first to arrive
    chunks["WallA"] = lay([
        ("s0T", F, NA), ("R", NA, 3),
        ("halfdmask", NA, NA), ("diagI", NA, NA), ("mub", K, 1),
        ("ones128inv", F, 1), ("ones1", 1, F), ("ones32", 1, NA),
    ])
    # bf16 fast-path PE operands
    enth = [("ones1h", 1, F), ("ones32h", 1, NA), ("HH", F, F)]
    for l in range(L):
        enth += [(f"Wdk{l}", K, F), (f"Wdv{l}", K, 3 * F)]
    chunks["WallH"] = lay(enth)
    # per-layer f32 node weights (folded LN affine + temperature)
    for l in range(L):
        chunks[f"W{l}"] = lay([
            (f"Wq{l}", F, F), (f"bqr{l}", 1, F),
            (f"Wk{l}", F, F), (f"bkr{l}", 1, F),
            (f"Wv{l}", F, 3 * F), (f"bvr{l}", 1, 3 * F),
            (f"Wo{l}", F, 3 * F), (f"bor{l}", 1, 3 * F),
            (f"U1{l}", F, F), (f"U2{l}", F, F), (f"U3{l}", F, F),
            (f"bdk{l}", F, 1), (f"bdv{l}", F, 3),
        ])
    chunks["WF"] = lay([("w1p", F, F // 2), ("b1r", 1, F // 2), ("w2", F // 2, 1)])
    return chunks


def _host_prep(inp):
    """Fold LN affine + temperature into weights; pack into chunked walls."""
    f32 = np.float32
    import ml_dtypes
    bf16 = ml_dtypes.bfloat16
    Z = np.asarray(inp["Z"]).reshape(-1)
    Rfull = np.asarray(inp["R"], f32).reshape(N_ATOM, 3)
    embed = np.asarray(inp["embed"], f32)
    s0 = embed[Z]                                   # (256, F) gather on host

    vals = {}
    for l in range(L):
        g = np.asarray(inp["ln_g"][l], f32)
        b = np.asarray(inp["ln_b"][l], f32)
        Wq = np.asarray(inp["Wq"][l], f32)
        Wk = np.asarray(inp["Wk"][l], f32)
        Wv = np.asarray(inp["Wv"][l], f32)
        vals[f"Wq{l}"] = g[:, None] * Wq / TEMPERATURE
        vals[f"bqr{l}"] = (b @ Wq / TEMPERATURE).reshape(1, F)
        vals[f"Wk{l}"] = g[:, None] * Wk
        vals[f"bkr{l}"] = (b @ Wk).reshape(1, F)
        vals[f"Wv{l}"] = g[:, None] * Wv
        vals[f"bvr{l}"] = (b @ Wv).reshape(1, 3 * F)
        vals[f"Wdk{l}"] = np.asarray(inp["Wdk"][l], f32)
        vals[f"bdk{l}"] = np.asarray(inp["bdk"][l], f32).reshape(F, 1)
        vals[f"Wdv{l}"] = np.asarray(inp["Wdv"][l], f32)
        vals[f"bdv{l}"] = np.asarray(inp["bdv"][l], f32).reshape(3, F).T
        vals[f"Wo{l}"] = np.asarray(inp["Wo"][l], f32)
        vals[f"bor{l}"] = np.asarray(inp["bo"][l], f32).reshape(1, 3 * F)
        vals[f"U1{l}"] = np.asarray(inp["U1"][l], f32)
        vals[f"U2{l}"] = np.asarray(inp["U2"][l], f32)
        vals[f"U3{l}"] = np.asarray(inp["U3"][l], f32)

    lg = np.asarray(inp["lnf_g"], f32)
    lb = np.asarray(inp["lnf_b"], f32)
    w1 = np.asarray(inp["out_w1"], f32)
    vals["w1p"] = lg[:, None] * w1
    vals["b1r"] = (lb @ w1 + np.asarray(inp["out_b1"], f32)).reshape(1, F // 2)
    vals["w2"] = np.asarray(inp["out_w2"], f32).reshape(F // 2, 1)

    hh = np.zeros((F, F), f32)
    for h in range(H):
        hh[h * Dh:(h + 1) * Dh, h * Dh:(h + 1) * Dh] = 1.0
    vals["HH"] = hh
    eye = np.eye(NA, dtype=f32)
    vals["halfdmask"] = (0.5 * (1.0 - eye)).astype(f32)
    vals["diagI"] = eye
    mu = np.linspace(0.0, RC, K).astype(f32)
    vals["mub"] = (-np.sqrt(GAMMA) * mu).reshape(K, 1).astype(f32)
    vals["ones128inv"] = np.full((F, 1), 1.0 / F, f32)
    vals["ones1"] = np.ones((1, F), f32)
    vals["ones1h"] = np.ones((1, F), f32)
    vals["ones32"] = np.ones((1, NA), f32)
    vals["ones32h"] = np.ones((1, NA), f32)

    chunks = _chunk_layouts()
    arrays = {}
    for cname, (offs, C) in chunks.items():
        dt = f32 if cname in ("WallA", "WF") else bf16
        arr = np.zeros((F, C), dtype=dt)
        for n, (c0, p, w) in offs.items():
            if cname == "WallA" and n in ("s0T", "R"):
                continue  # per-molecule, filled below
            arr[0:p, c0:c0 + w] = vals[n].astype(dt)
        arrays[cname] = np.ascontiguousarray(arr)

    offsA, CA = chunks["WallA"]
    wallAs = []
    for m in range(N_MOL):
        wl = arrays["WallA"].copy()
        c0, p, w = offsA["s0T"]
        wl[0:p, c0:c0 + w] = s0[m * NA:(m + 1) * NA].T
        c0, p, w = offsA["R"]
        wl[0:p, c0:c0 + w] = Rfull[m * NA:(m + 1) * NA]
        wallAs.append(np.ascontiguousarray(wl))
    b2 = float(np.asarray(inp["out_b2"]).reshape(-1)[0])
    return arrays, wallAs, b2


_CACHE = {}


def kernel(**inputs):
    from concourse import bass_utils

    arrays, wallAs, b2 = _host_prep(inputs)

    key = ("nc", b2)
    if key not in _CACHE:
        _CACHE[key] = _build(b2)
    nc = _CACHE[key]

    in_maps = []
    for m in range(N_MOL):
        d = {cn: arrays[cn] for cn in arrays if cn != "WallA"}
        d["WallA"] = wallAs[m]
        in_maps.append(d)
    res = bass_utils.run_bass_kernel_spmd(nc, in_maps, core_ids=list(range(N_MOL)))
    out = np.concatenate([r["energy"].reshape(1) for r in res.results]).reshape(N_MOL, 1)
    return out.astype(np.float32)


def _patch_tile_drain():
    """The Tile kernel-tail drain carries one sem-wait per active processor;
    this walrus build caps sync waits per CTRL instruction. Split the waits
    onto individual SP nops (same semantics: all run before the exit
    barrier on the sync engine)."""
    import concourse.tile as tile_mod
    import bass_rust
    from concourse.vector_clock import ScopedClock

    if getattr(tile_mod.TileContext, "_drain_split_patched", False):
        return

    def _drain_and_barrier(self, tick_clock, wait_clock):
        nc = self.nc
        drain_inst = nc.sync.drain()
        wait_clock.add_sem_waits(
            drain_inst.ins, ScopedClock({None: tick_clock.global_clock})
        )
        si = drain_inst.ins.sync_info
        waits = list(si.on_wait or []) if si is not None else []
        if len(waits) > 1:
            drain_inst.ins.sync_info = bass_rust.SyncInfo(
                on_wait=waits[:1], on_update=list(si.on_update or []))
            for w in waits[1:]:
                nop = nc.sync.nop(nofuse=True)
                nop.ins.sync_info = bass_rust.SyncInfo(on_wait=[w], on_update=[])
        nc.all_engine_barrier()
        popped = nc._tile_sem_poison_stack.pop()
        assert popped is self._sem_poison
        nc.clear_and_free_semaphores(list(self.sems.allocated().values()))
        nc.all_engine_barrier()

    tile_mod.TileContext._drain_and_barrier = _drain_and_barrier
    tile_mod.TileContext._drain_split_patched = True


def _split_sync_waits(nc, mybir):
    """This walrus build rejects instructions carrying more than one sync
    wait ("Too many sync wait commands"). Hoist extra waits onto inserted
    same-engine NoOps immediately before the instruction — the engine
    sequencer blocks on each in turn, preserving the happens-before."""
    import bass_rust

    n_split = 0
    for fn in nc.m.functions:
        for bb in fn.blocks:
            changed = False
            new = []
            for ins in bb.instructions:
                si = ins.sync_info
                waits = list(si.on_wait or []) if si is not None else []
                if len(waits) > 1:
                    for i, w in enumerate(waits[:-1]):
                        nop = mybir.InstNoOp(name=f"{ins.name}-sw{i}")
                        nop.engine = ins.engine
                        nop.sync_info = bass_rust.SyncInfo(on_wait=[w], on_update=[])
                        nc.inst_map[nop.name] = nop
                        new.append(nop)
                    ins.sync_info = bass_rust.SyncInfo(
                        on_wait=[waits[-1]], on_update=list(si.on_update or []))
                    changed = True
                    n_split += 1
                new.append(ins)
            if changed:
                bb.instructions = new
    return n_split


def _build(b2):
    import concourse.bass as bass
    import concourse.mybir as mybir
    import concourse.tile as tile

    _patch_tile_drain()

    f32 = mybir.dt.float32
    bf16 = mybir.dt.bfloat16
    AF = mybir.ActivationFunctionType
    ALU = mybir.AluOpType
    AX = mybir.AxisListType

    def bcast_inner(ap, outer, inner):
        # (P, n) -> (P, outer(step), inner(bcast)): value[p, i, j] = ap[p, i]
        return bass.AP(tensor=ap.tensor, offset=ap.offset,
                       ap=[ap.ap[0], [ap.ap[1][0], outer], [0, inner]])

    def bcast_outer(ap, outer, inner):
        # (P, n) -> (P, outer(bcast), inner(step)): value[p, i, j] = ap[p, j]
        return bass.AP(tensor=ap.tensor, offset=ap.offset,
                       ap=[ap.ap[0], [0, outer], [ap.ap[1][0], inner]])

    nc = bass.Bass()
    chunks = _chunk_layouts()
    dram = {}
    for cn, (offs, C) in chunks.items():
        dt = f32 if cn in ("WallA", "WF") else bf16
        dram[cn] = nc.dram_tensor(cn, [F, C], dt, kind="ExternalInput")
    energy = nc.dram_tensor("energy", [1, 1], f32, kind="ExternalOutput")

    with tile.TileContext(nc) as tc:
        with tc.tile_pool(name="const", bufs=1) as cp, \
             tc.tile_pool(name="geo", bufs=1) as gp, \
             tc.tile_pool(name="small", bufs=3) as sp, \
             tc.tile_pool(name="wide", bufs=10) as wp, \
             tc.tile_pool(name="psE", bufs=2, space="PSUM") as psE, \
             tc.tile_pool(name="psX", bufs=1, space="PSUM") as psX, \
             tc.tile_pool(name="psS", bufs=2, space="PSUM") as psS, \
             tc.tile_pool(name="psN", bufs=2, space="PSUM") as psN:

            # ---- chunked weight DMAs, in priority order ----
            walls = {}
            W = {}

            def load_chunk(cn):
                offs, C = chunks[cn]
                dt = f32 if cn in ("WallA", "WF") else bf16
                t = cp.tile([F, C], dt, tag=cn, name=cn)
                nc.sync.dma_start(out=t[:], in_=dram[cn][:])
                walls[cn] = t
                for n, (c0, p, w) in offs.items():
                    W[n] = t[0:p, c0:c0 + w]

            order = ["WallA", "W0", "WallH"]
            for cn in order:
                offs, C = chunks[cn]
                dt = f32 if cn in ("WallA", "WF") else bf16
                t = cp.tile([F, C], dt, tag=cn, name=cn)
                nc.sync.dma_start(out=t[:], in_=dram[cn][:])
                walls[cn] = t
                for n, (c0, p, w) in offs.items():
                    W[n] = t[0:p, c0:c0 + w]
                if cn == "WallA":
                    # geometry input: R replicated across partitions; must be
                    # early on the serial DMA-engines resource
                    Rb = gp.tile([NA, NA * 3], f32, tag="Rb")
                    rc0 = offs["R"][0]
                    CA = C
                    nc.sync.dma_start(
                        out=Rb[:], in_=bass.AP(tensor=dram[cn][:].tensor, offset=rc0,
                                               ap=[[0, NA], [CA, NA], [1, 3]]))

            # small constant bias tiles for ACT (only 0.0/1.0 have const APs)
            b30 = cp.tile([NA, 1], f32, tag="b30", name="b30")
            nc.vector.memset(b30[:], 1e-30)
            bpi2 = cp.tile([NA, 1], f32, tag="bpi2", name="bpi2")
            nc.vector.memset(bpi2[:], PI / 2)
            beps = cp.tile([1, 1], f32, tag="beps", name="beps")
            nc.vector.memset(beps[:], 1e-5)

            # persistent state (sT first: LN(s0) is on the spine immediately)
            sT = gp.tile([F, NA], f32, tag="sT")
            nc.vector.tensor_copy(sT[:], W["s0T"][:])
            oT = gp.tile([F, NA], f32, tag="oT")
            nc.vector.memset(oT[:], 0.0)
            vT = []
            for c in range(3):
                t = gp.tile([F, NA], f32, tag=f"vT{c}")
                nc.gpsimd.memset(t[:], 0.0)
                vT.append(t)

            # =========== geometry ===========
            V = gp.tile([NA, NA, 3], f32, tag="V")      # vec[a, b, c] = R[a,c] - R[b,c]
            Ra = W["R"][:]
            Ra_b = bass.AP(tensor=Ra.tensor, offset=Ra.offset,
                           ap=[Ra.ap[0], [0, NA], [Ra.ap[1][0], 3]])
            nc.vector.tensor_sub(V[:], Ra_b, Rb[:].rearrange("p (b c) -> p b c", c=3))
            V2 = sp.tile([NA, NA, 3], f32, tag="V2")
            nc.vector.tensor_mul(V2[:], V[:], V[:])
            d2 = sp.tile([NA, NA], f32, tag="d2")
            nc.vector.reduce_sum(d2[:], V2[:], axis=AX.X)
            lnd2 = sp.tile([NA, NA], f32, tag="lnd2")
            nc.scalar.activation(lnd2[:], d2[:], AF.Ln, bias=b30[:])
            dmat = gp.tile([NA, NA], f32, tag="dmat")   # d = exp(0.5*ln(d2))
            nc.scalar.activation(dmat[:], lnd2[:], AF.Exp, scale=0.5)
            dsafe = sp.tile([NA, NA], f32, tag="dsafe")
            nc.vector.tensor_add(dsafe[:], dmat[:], W["diagI"][:])
            invd = sp.tile([NA, NA], f32, tag="invd")
            nc.vector.reciprocal(invd[:], dsafe[:])
            vn = gp.tile([NA, NA, 3], f32, tag="vn")    # vec_norm (diag exactly 0)
            iap = invd[:]
            nc.vector.tensor_mul(vn[:], V[:], bass.AP(tensor=iap.tensor, offset=iap.offset,
                                                      ap=[iap.ap[0], [iap.ap[1][0], NA], [0, 3]]))
            vn_b = gp.tile([NA, NA, 3], bf16, tag="vn_b", name="vn_b")
            nc.vector.tensor_copy(vn_b[:], vn[:])

            # cutoff: co = 0.5*(cos(pi*d/RC)+1)*(d<=RC), diag zero
            stepm = sp.tile([NA, NA], f32, tag="stepm")
            nc.vector.tensor_scalar(out=stepm[:], in0=dmat[:], scalar1=RC, scalar2=None,
                                    op0=ALU.is_le)
            d_cl = sp.tile([NA, NA], f32, tag="d_cl")
            nc.vector.tensor_scalar(out=d_cl[:], in0=dmat[:], scalar1=RC, scalar2=None,
                                    op0=ALU.min)
            s1 = sp.tile([NA, NA], f32, tag="s1")
            nc.scalar.activation(s1[:], d_cl[:], AF.Sin, bias=bpi2[:], scale=-PI / RC)
            m32 = sp.tile([NA, NA], f32, tag="m32")
            nc.vector.tensor_mul(m32[:], stepm[:], W["halfdmask"][:])
            co_a = gp.tile([NA, NA], f32, tag="co_a")
            nc.vector.scalar_tensor_tensor(co_a[:], s1[:], 1.0, m32[:],
                                           op0=ALU.add, op1=ALU.mult)
            co_b = gp.tile([NA, NA], bf16, tag="co_b", name="co_b")
            nc.vector.tensor_copy(co_b[:], co_a[:])

            # row-ized [1, NE] copies (spread across DMA queues)
            dE = gp.tile([1, NE], f32, tag="dE", name="dE")
            nc.sync.dma_start(out=dE[:], in_=dmat[:])
            coE = gp.tile([1, NE], bf16, tag="coE", name="coE")
            nc.sync.dma_start(out=coE[:], in_=co_b[:])
            vnrow = [gp.tile([1, NE], bf16, tag=f"vnrow{c}", name=f"vnrow{c}")
                     for c in range(3)]
            nc.sync.dma_start(out=vnrow[0][:], in_=vn_b[:, :, 0])
            nc.sync.dma_start(out=vnrow[1][:], in_=vn_b[:, :, 1])
            nc.sync.dma_start(out=vnrow[2][:], in_=vn_b[:, :, 2])
            # late weight chunks: gate each behind the last rowize DMA so they
            # cannot grab the serial DMA engines before the geometry rows land
            for cn in ["W1", "W2", "W3", "WF"]:
                offs, C = chunks[cn]
                dt = f32 if cn in ("WallA", "WF") else bf16
                t = cp.tile([F, C], dt, tag=cn, name=cn)
                nc.vector.tensor_copy(t[0:1, 0:1], vnrow[2][0:1, 0:1])
                nc.sync.dma_start(out=t[:], in_=dram[cn][:])
                walls[cn] = t
                for n, (c0, p, w) in offs.items():
                    W[n] = t[0:p, c0:c0 + w]

            # RBF edge features erbf[K, NE] = exp(-gamma (d - mu)^2), bf16
            erbf = gp.tile([K, NE], bf16, tag="erbf", name="erbf")
            for hh_ in range(2):
                sl = slice(hh_ * 512, (hh_ + 1) * 512)
                pb = psE.tile([K, 512], f32, tag="edge", name="pb_rbf")
                nc.tensor.matmul(pb[:], W["ones1"][0:1, 0:K], dE[:, sl],
                                 start=True, stop=True)
                sq = sp.tile([K, 512], f32, tag="rbf_sq")
                nc.scalar.activation(sq[:], pb[:], AF.Square, bias=W["mub"][:],
                                     scale=float(np.sqrt(GAMMA)))
                nc.scalar.activation(erbf[:, sl], sq[:], AF.Exp, scale=-1.0)

            # broadcast co50 [K,NE] now (feeds e_full); co128/vnE are deferred
            # until after layer-0's edge MLP so they don't block its silus
            co50 = gp.tile([K, NE], bf16, tag="co50")
            for hh_ in range(2):
                sl = slice(hh_ * 512, (hh_ + 1) * 512)
                pb = psE.tile([K, 512], f32, tag="edge", name="pb_co50")
                nc.tensor.matmul(pb[:], W["ones1h"][0:1, 0:K], coE[:, sl],
                                 start=True, stop=True)
                nc.scalar.copy(co50[:, sl], pb[:])

            def bcast_co_vne():
                co128 = gp.tile([F, NE], bf16, tag="co128")
                for hh_ in range(2):
                    sl = slice(hh_ * 512, (hh_ + 1) * 512)
                    pb2 = psE.tile([F, 512], f32, tag="edge", name="pb_co128")
                    nc.tensor.matmul(pb2[:], W["ones1h"][:], coE[:, sl],
                                     start=True, stop=True)
                    nc.scalar.copy(co128[:, sl], pb2[:])
                vnE = []
                for c in range(3):
                    t = gp.tile([F, NE], bf16, tag=f"vnE{c}", name=f"vnE{c}")
                    for hh_ in range(2):
                        sl = slice(hh_ * 512, (hh_ + 1) * 512)
                        pb = psE.tile([F, 512], f32, tag="edge", name="pb_vne")
                        nc.tensor.matmul(pb[:], W["ones1h"][:], vnrow[c][:, sl],
                                         start=True, stop=True)
                        if c == 2:
                            nc.scalar.copy(t[:, sl], pb[:])
                        else:
                            nc.vector.tensor_copy(t[:, sl], pb[:])
                    vnE.append(t)
                return co128, vnE

            e_full = gp.tile([K, NE], bf16, tag="e_full", name="e_full")
            nc.vector.tensor_mul(e_full[:], erbf[:], co50[:])

            def e3(t):
                return t[:].rearrange("p (a b) -> p a b", a=NA)

            # edge MLP for one layer: dk + dv1/2/3 with co folded into dv
            def edge_mlp(l):
                dk = gp.tile([F, NE], bf16, tag=f"dk{l}")
                for h in range(2):
                    sl = slice(h * 512, (h + 1) * 512)
                    pm = psE.tile([F, 512], f32, tag="edge")
                    nc.tensor.matmul(pm[:], W[f"Wdk{l}"], e_full[:, sl],
                                     start=True, stop=True)
                    nc.scalar.activation(dk[:, sl], pm[:], AF.Silu, bias=W[f"bdk{l}"][:])
                dvl = []
                for c in range(3):
                    dv = gp.tile([F, NE], bf16, tag=f"dv{l}_{c}")
                    for h in range(2):
                        sl = slice(h * 512, (h + 1) * 512)
                        pm = psE.tile([F, 512], f32, tag="edge")
                        nc.tensor.matmul(pm[:], W[f"Wdv{l}"][:, c * F:(c + 1) * F],
                                         e_full[:, sl], start=True, stop=True)
                        nc.scalar.activation(dv[:, sl], pm[:], AF.Silu,
                                             bias=W[f"bdv{l}"][:, c:c + 1])
                    dvl.append(dv)
                return dk, dvl

            def layernorm_f(inT):
                # LN stats over the feature (partition) axis via PE ones-matmuls
                sq = sp.tile([F, NA], f32, tag="lnsq")
                nc.scalar.activation(sq[:], inT[:], AF.Square)
                stat = psN.tile([1, 2 * NA], f32, tag="nst")
                nc.tensor.matmul(stat[:, 0:NA], W["ones128inv"][:], inT[:],
                                 start=True, stop=True)
                nc.tensor.matmul(stat[:, NA:2 * NA], W["ones128inv"][:], sq[:],
                                 start=True, stop=True)
                statm = stat[:, 0:NA]
                varr = sp.tile([1, NA], f32, tag="varr")
                nc.scalar.activation(varr[:], statm, AF.Square)
                nc.vector.tensor_sub(varr[:], stat[:, NA:2 * NA], varr[:])
                rb = sp.tile([1, 2 * NA], f32, tag="rb")
                lnv = sp.tile([1, NA], f32, tag="lnv")
                nc.scalar.activation(lnv[:], varr[:], AF.Ln, bias=beps[:])
                nc.scalar.activation(rb[:, 0:NA], lnv[:], AF.Exp, scale=-0.5)   # rstd
                nc.vector.tensor_mul(rb[:, NA:2 * NA], statm, rb[:, 0:NA])      # mu*rstd
                bc = psN.tile([F, 2 * NA], f32, tag="nst")
                nc.tensor.matmul(bc[:], W["ones1"][:], rb[:], start=True, stop=True)
                xh = sp.tile([F, NA], f32, tag="xhatT")
                nc.vector.tensor_mul(xh[:], inT[:], bc[:, 0:NA])
                nc.vector.tensor_sub(xh[:], xh[:], bc[:, NA:2 * NA])
                return xh

            # layer-0 edge MLP emitted before the loop so ACT starts early
            dks, dvs = [None] * L, [None] * L
            dks[0], dvs[0] = edge_mlp(0)
            co128, vnE = bcast_co_vne()

            # =========== interaction layers ===========
            for l in range(L):
                first = l == 0

                # LN first: its PE stats must not queue behind the U-matmuls
                # (which wait on late vT updates from the previous layer)
                xhatT = layernorm_f(sT)
                xhb = sp.tile([F, NA], bf16, tag="xhb")
                nc.vector.tensor_copy(xhb[:], xhatT[:])

                # U-matmuls + dot chain: need only prev-layer vT; overlap LN
                ou = psS.tile([F, 512], f32, tag="qkv")
                if not first:
                    vTb = []
                    for c in range(3):
                        vb = sp.tile([F, NA], bf16, tag=f"vTb{c}")
                        nc.gpsimd.tensor_copy(vb[:], vT[c][:])
                        vTb.append(vb)
                    for c in range(3):
                        nc.tensor.matmul(ou[:, (3 + c) * NA:(4 + c) * NA], W[f"U1{l}"][:],
                                         vTb[c][:], start=True, stop=True)
                        nc.tensor.matmul(ou[:, (6 + c) * NA:(7 + c) * NA], W[f"U2{l}"][:],
                                         vTb[c][:], start=True, stop=True)
                        nc.tensor.matmul(ou[:, (9 + c) * NA:(10 + c) * NA], W[f"U3{l}"][:],
                                         vTb[c][:], start=True, stop=True)
                    us = sp.tile([F, 9 * NA], f32, tag="us")
                    nc.vector.tensor_copy(us[:], ou[:, 3 * NA:12 * NA])
                    dot = sp.tile([F, NA], f32, tag="dot")
                    pc = sp.tile([F, NA], f32, tag="dotp")
                    nc.gpsimd.tensor_mul(dot[:], us[:, 0:NA], us[:, 3 * NA:4 * NA])
                    for c in range(1, 3):
                        nc.gpsimd.tensor_mul(pc[:], us[:, c * NA:(c + 1) * NA],
                                             us[:, (3 + c) * NA:(4 + c) * NA])
                        nc.gpsimd.tensor_add(dot[:], dot[:], pc[:])

                # node matmuls with PE-accumulated biases; all stay in PSUM
                qkv = psS.tile([F, 512], f32, tag="qkv")

                def node_mm(dst, wap, brow):
                    nc.tensor.matmul(dst, brow, W["ones32h"][:], start=True, stop=False)
                    nc.tensor.matmul(dst, wap, xhb[:], start=False, stop=True)

                node_mm(qkv[:, 0:NA], W[f"Wq{l}"][:], W[f"bqr{l}"][:])
                node_mm(qkv[:, NA:2 * NA], W[f"Wk{l}"][:], W[f"bkr{l}"][:])
                for c in range(3):
                    node_mm(qkv[:, (2 + c) * NA:(3 + c) * NA],
                            W[f"Wv{l}"][:, c * F:(c + 1) * F],
                            W[f"bvr{l}"][:, c * F:(c + 1) * F])
                q_ap = qkv[:, 0:NA]
                kb = sp.tile([F, NA], bf16, tag="kb")
                nc.vector.tensor_copy(kb[:], qkv[:, NA:2 * NA])
                k_ap = kb[:]
                val = [qkv[:, (2 + c) * NA:(3 + c) * NA] for c in range(3)]
                # bf16 copies + G on Pool (off the DVE spine)
                val1b = sp.tile([F, NA], bf16, tag="val1b")
                nc.vector.tensor_copy(val1b[:], val[0])
                val3b = sp.tile([F, NA], bf16, tag="val3b")
                nc.vector.tensor_copy(val3b[:], val[2])
                if not first:
                    val2b = sp.tile([F, NA], bf16, tag="val2b")
                    nc.vector.tensor_copy(val2b[:], val[1])
                    G = []
                    for c in range(3):
                        g = sp.tile([F, NA], bf16, tag=f"G{c}")
                        nc.gpsimd.tensor_mul(g[:], val2b[:], vT[c][:])
                        G.append(g)

                # logits: qk = q (x) k; prod = qk * dk; head-sum via HH; exp
                qk = wp.tile([F, NA, NA], bf16, tag="w")
                prod = wp.tile([F, NA, NA], bf16, tag="w")
                Xp = wp.tile([F, NE], bf16, tag="w")
                lps = psX.tile([F, NE], f32, tag="lg")
                for hch in range(2):
                    asl = slice(hch * 16, (hch + 1) * 16)      # a-halves
                    csl = slice(hch * 512, (hch + 1) * 512)
                    nc.vector.tensor_mul(qk[:, asl, :], bcast_inner(q_ap[:, asl], 16, NA),
                                         bcast_outer(k_ap, 16, NA))
                    nc.vector.tensor_mul(prod[:, asl, :], qk[:, asl, :],
                                         e3(dks[l])[:, asl, :])
                    nc.tensor.matmul(lps[:, csl],
                                     W["HH"], prod[:].rearrange("p a b -> p (a b)")[:, csl],
                                     start=True, stop=True)
                    nc.scalar.activation(Xp[:, csl], lps[:, csl], AF.Exp)

                # W1 = dv1 * val1b rides in the DVE gap while PE/ACT do exp
                W1c = wp.tile([F, NA, NA], bf16, tag="w")
                nc.vector.tensor_mul(W1c[:], e3(dvs[l][0]), bcast_outer(val1b[:], NA, NA))

                # edge MLP for the next layer rides behind this layer's ACT work
                if l + 1 < L:
                    dks[l + 1], dvs[l + 1] = edge_mlp(l + 1)

                # spine: S -> invD -> Y -> P1v -> ds -> Wo -> dx -> sT
                S = sp.tile([F, NA], f32, tag="S")
                nc.vector.reduce_sum(S[:, 0:16], e3(Xp)[:, 0:16, :], axis=AX.X)
                nc.vector.reduce_sum(S[:, 16:NA], e3(Xp)[:, 16:NA, :], axis=AX.X)
                xap = Xp[:]
                diag_ap = bass.AP(tensor=xap.tensor, offset=xap.offset,
                                  ap=[xap.ap[0], [(NA + 1) * xap.ap[1][0], NA]])
                invD = sp.tile([F, NA], f32, tag="invD")
                nc.vector.tensor_sub(invD[:], S[:], diag_ap)
                nc.vector.reciprocal(invD[:], invD[:])
                Y = wp.tile([F, NA, NA], bf16, tag="w")
                P1v = wp.tile([F, NA, NA], bf16, tag="w")
                dsT = sp.tile([F, NA], f32, tag="dsT")
                for hch in range(2):
                    asl = slice(hch * 16, (hch + 1) * 16)
                    nc.vector.tensor_mul(Y[:, asl, :], e3(Xp)[:, asl, :],
                                         e3(co128)[:, asl, :])
                    nc.vector.tensor_mul(P1v[:, asl, :], Y[:, asl, :], W1c[:, asl, :])
                    nc.vector.reduce_sum(dsT[:, asl], P1v[:, asl, :], axis=AX.X)
                dsb = sp.tile([F, NA], bf16, tag="dsb")
                nc.vector.tensor_mul(dsb[:], dsT[:], invD[:])

                def node_mm2(dst, wap, brow, mov):
                    nc.tensor.matmul(dst, brow, W["ones32h"][:], start=True, stop=False)
                    nc.tensor.matmul(dst, wap, mov, start=False, stop=True)

                for c in range(3):
                    node_mm2(ou[:, c * NA:(c + 1) * NA], W[f"Wo{l}"][:, c * F:(c + 1) * F],
                             W[f"bor{l}"][:, c * F:(c + 1) * F], dsb[:])
                o1, o2, o3 = (ou[:, c * NA:(c + 1) * NA] for c in range(3))
                dx = sp.tile([F, NA], f32, tag="dx")
                if first:
                    nc.vector.tensor_copy(dx[:], o2)
                else:
                    nc.vector.tensor_mul(dx[:], o3, dot[:])
                    nc.vector.tensor_add(dx[:], dx[:], o2)
                nc.vector.tensor_add(sT[:], sT[:], dx[:])
                nc.vector.tensor_add(oT[:], oT[:], dx[:])

                # dw messages trail the spine; only next layer's U/G need them
                X3 = wp.tile([F, NA, NA], bf16, tag="w")
                nc.vector.tensor_mul(X3[:], Y[:], bcast_outer(val3b[:], NA, NA))
                P3 = wp.tile([F, NA, NA], bf16, tag="w")
                nc.vector.tensor_mul(P3[:], X3[:], e3(dvs[l][2]))
                if not first:
                    P2 = wp.tile([F, NA, NA], bf16, tag="w")
                    nc.vector.tensor_mul(P2[:], Y[:], e3(dvs[l][1]))
                dwm = sp.tile([F, 3, NA], f32, tag="dwm")
                for c in range(3):
                    tt = wp.tile([F, NA, NA], bf16, tag="w")
                    if c == 2:
                        nc.gpsimd.tensor_mul(tt[:], P3[:], e3(vnE[c]))
                    else:
                        nc.vector.tensor_mul(tt[:], P3[:], e3(vnE[c]))
                    if not first:
                        rr = wp.tile([F, NA, NA], bf16, tag="w")
                        if c == 2:
                            nc.gpsimd.tensor_mul(rr[:], P2[:], bcast_outer(G[c][:], NA, NA))
                            nc.vector.tensor_add(tt[:], tt[:], rr[:])
                        else:
                            nc.vector.tensor_mul(rr[:], P2[:], bcast_outer(G[c][:], NA, NA))
                            nc.vector.tensor_add(tt[:], tt[:], rr[:])
                    nc.vector.reduce_sum(dwm[:, c, :], tt[:], axis=AX.X)
                iap2 = invD[:]
                nc.vector.tensor_mul(dwm[:], dwm[:],
                                     bass.AP(tensor=iap2.tensor, offset=iap2.offset,
                                             ap=[iap2.ap[0], [0, 3], [iap2.ap[1][0], NA]]))
                dwT = [dwm[:, c, :] for c in range(3)]
                if first:
                    for c in range(3):
                        nc.gpsimd.tensor_copy(vT[c][:], dwT[c])
                else:
                    o1s = sp.tile([F, NA], f32, tag="o1s")
                    nc.vector.tensor_copy(o1s[:], o1)
                    for c in range(3):
                        t3 = sp.tile([F, NA], f32, tag="t3")
                        nc.gpsimd.tensor_mul(t3[:], o1s[:], us[:, (6 + c) * NA:(7 + c) * NA])
                        nc.gpsimd.tensor_add(vT[c][:], vT[c][:], dwT[c])
                        nc.gpsimd.tensor_add(vT[c][:], vT[c][:], t3[:])

            # =========== final LN + output MLP ===========
            xo = layernorm_f(oT)
            y_p = psS.tile([F // 2, NA], f32, tag="qkv")
            nc.tensor.matmul(y_p[:], W["b1r"][:], W["ones32"][:], start=True, stop=False)
            nc.tensor.matmul(y_p[:], W["w1p"][:], xo[:], start=False, stop=True)
            a1 = sp.tile([F // 2, NA], f32, tag="a1")
            nc.scalar.activation(a1[:], y_p[:], AF.Silu)
            asum = sp.tile([F // 2, 1], f32, tag="asum")
            nc.vector.reduce_sum(asum[:], a1[:], axis=AX.X)
            en_p = psS.tile([1, 1], f32, tag="qkv")
            nc.tensor.matmul(en_p[:], W["w2"][:], asum[:], start=True, stop=True)
            en = sp.tile([1, 1], f32, tag="en")
            nc.vector.tensor_scalar(out=en[:], in0=en_p[:], scalar1=float(NA * b2),
                                    scalar2=None, op0=ALU.add)
            nc.sync.dma_start(out=energy[:], in_=en[:])

    _split_sync_waits(nc, mybir)
    nc.finalize()
    return nc


# revision 21
# speedup vs baseline: 1.0440x; 1.0147x over previous
"""CMRET equivariant message-passing GNN — Trainium2 Bass kernel.

One molecule per NeuronCore (the batch mask is block-diagonal: 8 molecules x
32 contiguous atoms; every pairwise term is zero across molecules), dense
32x32 local attention per molecule, no collectives.

v2 layout/schedule:
- Weights stream in ordered DMA chunks (consts+s0 first, per-layer node
  weights next) so compute starts immediately and the 3MB weight load hides
  under the first ~10us of compute.
- Edge MLPs (dk/dv silu tensors, static per layer) are emitted interleaved
  with the layer loop so ACT works on layer l+1's silus while DVE/Pool crunch
  layer l's messages.
- The cosine cutoff is folded into dv1/dv2/dv3 once per layer (they are
  static), so messages use Xp = exp(logits) directly; no Ec/lnco machinery.
- All node-matmul biases are accumulated in PSUM via bias-row x ones-row
  matmuls; downstream engines read PSUM directly (no ACT bias passes).
- Per-edge products run on DVE in bf16 (2x mode); segmented b-reduces are
  split between DVE and the otherwise idle Pool (gpsimd) engine.
"""

import numpy as np

RC = 5.0
N_ATOM = 256
N_MOL = 8
NA = 32          # atoms per molecule
F = 128
K = 50
L = 4
H = 4
Dh = 32
TEMP = 2.0
NE = NA * NA     # dense per-molecule edges (diag masked)
GAMMA = 0.5 / (RC / (K - 1)) ** 2
TEMPERATURE = TEMP * np.sqrt(Dh)
PI = float(np.pi)


def _chunk_layouts():
    """DMA chunks, each a packed [p, w] block list: name -> (col0, parts, w)."""
    chunks = {}

    def lay(entries):
        offs, c = {}, 0
        for n, p, w in entries:
            offs[n] = (c, p, w)
            c += w
        return offs, c

    # consts + per-molecule state; first to arrive
    chunks["WallA"] = lay([
        ("s0T", F, NA), ("R", NA, 3),
        ("halfdmask", NA, NA), ("diagI", NA, NA), ("mub", K, 1),
        ("ones128inv", F, 1), ("ones1", 1, F), ("ones32", 1, NA),
    ])
    # bf16 fast-path PE operands
    enth = [("ones1h", 1, F), ("ones32h", 1, NA), ("HH", F, F)]
    for l in range(L):
        enth += [(f"Wdk{l}", K, F), (f"Wdv{l}", K, 3 * F)]
    chunks["WallH"] = lay(enth)
    # per-layer f32 node weights (folded LN affine + temperature)
    for l in range(L):
        chunks[f"W{l}"] = lay([
            (f"Wq{l}", F, F), (f"bqr{l}", 1, F),
            (f"Wk{l}", F, F), (f"bkr{l}", 1, F),
            (f"Wv{l}", F, 3 * F), (f"bvr{l}", 1, 3 * F),
            (f"Wo{l}", F, 3 * F), (f"bor{l}", 1, 3 * F),
            (f"U1{l}", F, F), (f"U2{l}", F, F), (f"U3{l}", F, F),
            (f"bdk{l}", F, 1), (f"bdv{l}", F, 3),
        ])
    chunks["WF"] = lay([("w1p", F, F // 2), ("b1r", 1, F // 2), ("w2", F // 2, 1)])
    return chunks


def _host_prep(inp):
    """Fold LN affine + temperature into weights; pack into chunked walls."""
    f32 = np.float32
    import ml_dtypes
    bf16 = ml_dtypes.bfloat16
    Z = np.asarray(inp["Z"]).reshape(-1)
    Rfull = np.asarray(inp["R"], f32).reshape(N_ATOM, 3)
    embed = np.asarray(inp["embed"], f32)
    s0 = embed[Z]                                   # (256, F) gather on host

    vals = {}
    for l in range(L):
        g = np.asarray(inp["ln_g"][l], f32)
        b = np.asarray(inp["ln_b"][l], f32)
        Wq = np.asarray(inp["Wq"][l], f32)
        Wk = np.asarray(inp["Wk"][l], f32)
        Wv = np.asarray(inp["Wv"][l], f32)
        vals[f"Wq{l}"] = g[:, None] * Wq / TEMPERATURE
        vals[f"bqr{l}"] = (b @ Wq / TEMPERATURE).reshape(1, F)
        vals[f"Wk{l}"] = g[:, None] * Wk
        vals[f"bkr{l}"] = (b @ Wk).reshape(1, F)
        vals[f"Wv{l}"] = g[:, None] * Wv
        vals[f"bvr{l}"] = (b @ Wv).reshape(1, 3 * F)
        vals[f"Wdk{l}"] = np.asarray(inp["Wdk"][l], f32)
        vals[f"bdk{l}"] = np.asarray(inp["bdk"][l], f32).reshape(F, 1)
        vals[f"Wdv{l}"] = np.asarray(inp["Wdv"][l], f32)
        vals[f"bdv{l}"] = np.asarray(inp["bdv"][l], f32).reshape(3, F).T
        vals[f"Wo{l}"] = np.asarray(inp["Wo"][l], f32)
        vals[f"bor{l}"] = np.asarray(inp["bo"][l], f32).reshape(1, 3 * F)
        vals[f"U1{l}"] = np.asarray(inp["U1"][l], f32)
        vals[f"U2{l}"] = np.asarray(inp["U2"][l], f32)
        vals[f"U3{l}"] = np.asarray(inp["U3"][l], f32)

    lg = np.asarray(inp["lnf_g"], f32)
    lb = np.asarray(inp["lnf_b"], f32)
    w1 = np.asarray(inp["out_w1"], f32)
    vals["w1p"] = lg[:, None] * w1
    vals["b1r"] = (lb @ w1 + np.asarray(inp["out_b1"], f32)).reshape(1, F // 2)
    vals["w2"] = np.asarray(inp["out_w2"], f32).reshape(F // 2, 1)

    hh = np.zeros((F, F), f32)
    for h in range(H):
        hh[h * Dh:(h + 1) * Dh, h * Dh:(h + 1) * Dh] = 1.0
    vals["HH"] = hh
    eye = np.eye(NA, dtype=f32)
    vals["halfdmask"] = (0.5 * (1.0 - eye)).astype(f32)
    vals["diagI"] = eye
    mu = np.linspace(0.0, RC, K).astype(f32)
    vals["mub"] = (-np.sqrt(GAMMA) * mu).reshape(K, 1).astype(f32)
    vals["ones128inv"] = np.full((F, 1), 1.0 / F, f32)
    vals["ones1"] = np.ones((1, F), f32)
    vals["ones1h"] = np.ones((1, F), f32)
    vals["ones32"] = np.ones((1, NA), f32)
    vals["ones32h"] = np.ones((1, NA), f32)

    chunks = _chunk_layouts()
    arrays = {}
    for cname, (offs, C) in chunks.items():
        dt = f32 if cname in ("WallA", "WF") else bf16
        arr = np.zeros((F, C), dtype=dt)
        for n, (c0, p, w) in offs.items():
            if cname == "WallA" and n in ("s0T", "R"):
                continue  # per-molecule, filled below
            arr[0:p, c0:c0 + w] = vals[n].astype(dt)
        arrays[cname] = np.ascontiguousarray(arr)

    offsA, CA = chunks["WallA"]
    wallAs = []
    for m in range(N_MOL):
        wl = arrays["WallA"].copy()
        c0, p, w = offsA["s0T"]
        wl[0:p, c0:c0 + w] = s0[m * NA:(m + 1) * NA].T
        c0, p, w = offsA["R"]
        wl[0:p, c0:c0 + w] = Rfull[m * NA:(m + 1) * NA]
        wallAs.append(np.ascontiguousarray(wl))
    b2 = float(np.asarray(inp["out_b2"]).reshape(-1)[0])
    return arrays, wallAs, b2


_CACHE = {}


def kernel(**inputs):
    from concourse import bass_utils

    arrays, wallAs, b2 = _host_prep(inputs)

    key = ("nc", b2)
    if key not in _CACHE:
        _CACHE[key] = _build(b2)
    nc = _CACHE[key]

    in_maps = []
    for m in range(N_MOL):
        d = {cn: arrays[cn] for cn in arrays if cn != "WallA"}
        d["WallA"] = wallAs[m]
        in_maps.append(d)
    res = bass_utils.run_bass_kernel_spmd(nc, in_maps, core_ids=list(range(N_MOL)))
    out = np.concatenate([r["energy"].reshape(1) for r in res.results]).reshape(N_MOL, 1)
    return out.astype(np.float32)


def _patch_tile_drain():
    """The Tile kernel-tail drain carries one sem-wait per active processor;
    this walrus build caps sync waits per CTRL instruction. Split the waits
    onto individual SP nops (same semantics: all run before the exit
    barrier on the sync engine)."""
    import concourse.tile as tile_mod
    import bass_rust
    from concourse.vector_clock import ScopedClock

    if getattr(tile_mod.TileContext, "_drain_split_patched", False):
        return

    def _drain_and_barrier(self, tick_clock, wait_clock):
        nc = self.nc
        drain_inst = nc.sync.drain()
        wait_clock.add_sem_waits(
            drain_inst.ins, ScopedClock({None: tick_clock.global_clock})
        )
        si = drain_inst.ins.sync_info
        waits = list(si.on_wait or []) if si is not None else []
        if len(waits) > 1:
            drain_inst.ins.sync_info = bass_rust.SyncInfo(
                on_wait=waits[:1], on_update=list(si.on_update or []))
            for w in waits[1:]:
                nop = nc.sync.nop(nofuse=True)
                nop.ins.sync_info = bass_rust.SyncInfo(on_wait=[w], on_update=[])
        nc.all_engine_barrier()
        popped = nc._tile_sem_poison_stack.pop()
        assert popped is self._sem_poison
        nc.clear_and_free_semaphores(list(self.sems.allocated().values()))
        nc.all_engine_barrier()

    tile_mod.TileContext._drain_and_barrier = _drain_and_barrier
    tile_mod.TileContext._drain_split_patched = True


def _split_sync_waits(nc, mybir):
    """This walrus build rejects instructions carrying more than one sync
    wait ("Too many sync wait commands"). Hoist extra waits onto inserted
    same-engine NoOps immediately before the instruction — the engine
    sequencer blocks on each in turn, preserving the happens-before."""
    import bass_rust

    n_split = 0
    for fn in nc.m.functions:
        for bb in fn.blocks:
            changed = False
            new = []
            for ins in bb.instructions:
                si = ins.sync_info
                waits = list(si.on_wait or []) if si is not None else []
                if len(waits) > 1:
                    for i, w in enumerate(waits[:-1]):
                        nop = mybir.InstNoOp(name=f"{ins.name}-sw{i}")
                        nop.engine = ins.engine
                        nop.sync_info = bass_rust.SyncInfo(on_wait=[w], on_update=[])
                        nc.inst_map[nop.name] = nop
                        new.append(nop)
                    ins.sync_info = bass_rust.SyncInfo(
                        on_wait=[waits[-1]], on_update=list(si.on_update or []))
                    changed = True
                    n_split += 1
                new.append(ins)
            if changed:
                bb.instructions = new
    return n_split


def _build(b2):
    import concourse.bass as bass
    import concourse.mybir as mybir
    import concourse.tile as tile

    _patch_tile_drain()

    f32 = mybir.dt.float32
    bf16 = mybir.dt.bfloat16
    AF = mybir.ActivationFunctionType
    ALU = mybir.AluOpType
    AX = mybir.AxisListType

    def bcast_inner(ap, outer, inner):
        # (P, n) -> (P, outer(step), inner(bcast)): value[p, i, j] = ap[p, i]
        return bass.AP(tensor=ap.tensor, offset=ap.offset,
                       ap=[ap.ap[0], [ap.ap[1][0], outer], [0, inner]])

    def bcast_outer(ap, outer, inner):
        # (P, n) -> (P, outer(bcast), inner(step)): value[p, i, j] = ap[p, j]
        return bass.AP(tensor=ap.tensor, offset=ap.offset,
                       ap=[ap.ap[0], [0, outer], [ap.ap[1][0], inner]])

    nc = bass.Bass()
    chunks = _chunk_layouts()
    dram = {}
    for cn, (offs, C) in chunks.items():
        dt = f32 if cn in ("WallA", "WF") else bf16
        dram[cn] = nc.dram_tensor(cn, [F, C], dt, kind="ExternalInput")
    energy = nc.dram_tensor("energy", [1, 1], f32, kind="ExternalOutput")

    with tile.TileContext(nc) as tc:
        with tc.tile_pool(name="const", bufs=1) as cp, \
             tc.tile_pool(name="geo", bufs=1) as gp, \
             tc.tile_pool(name="small", bufs=3) as sp, \
             tc.tile_pool(name="wide", bufs=10) as wp, \
             tc.tile_pool(name="psE", bufs=2, space="PSUM") as psE, \
             tc.tile_pool(name="psX", bufs=1, space="PSUM") as psX, \
             tc.tile_pool(name="psS", bufs=2, space="PSUM") as psS, \
             tc.tile_pool(name="psN", bufs=2, space="PSUM") as psN:

            # ---- chunked weight DMAs, in priority order ----
            walls = {}
            W = {}

            def load_chunk(cn):
                offs, C = chunks[cn]
                dt = f32 if cn in ("WallA", "WF") else bf16
                t = cp.tile([F, C], dt, tag=cn, name=cn)
                nc.sync.dma_start(out=t[:], in_=dram[cn][:])
                walls[cn] = t
                for n, (c0, p, w) in offs.items():
                    W[n] = t[0:p, c0:c0 + w]

            order = ["WallA", "W0", "WallH"]
            for cn in order:
                offs, C = chunks[cn]
                dt = f32 if cn in ("WallA", "WF") else bf16
                t = cp.tile([F, C], dt, tag=cn, name=cn)
                nc.sync.dma_start(out=t[:], in_=dram[cn][:])
                walls[cn] = t
                for n, (c0, p, w) in offs.items():
                    W[n] = t[0:p, c0:c0 + w]
                if cn == "WallA":
                    # geometry input: R replicated across partitions; must be
                    # early on the serial DMA-engines resource
                    Rb = gp.tile([NA, NA * 3], f32, tag="Rb")
                    rc0 = offs["R"][0]
                    CA = C
                    nc.sync.dma_start(
                        out=Rb[:], in_=bass.AP(tensor=dram[cn][:].tensor, offset=rc0,
                                               ap=[[0, NA], [CA, NA], [1, 3]]))

            # small constant bias tiles for ACT (only 0.0/1.0 have const APs)
            b30 = cp.tile([NA, 1], f32, tag="b30", name="b30")
            nc.vector.memset(b30[:], 1e-30)
            bpi2 = cp.tile([NA, 1], f32, tag="bpi2", name="bpi2")
            nc.vector.memset(bpi2[:], PI / 2)
            beps = cp.tile([1, 1], f32, tag="beps", name="beps")
            nc.vector.memset(beps[:], 1e-5)

            # persistent state (sT first: LN(s0) is on the spine immediately)
            sT = gp.tile([F, NA], f32, tag="sT")
            nc.vector.tensor_copy(sT[:], W["s0T"][:])
            oT = gp.tile([F, NA], f32, tag="oT")
            nc.vector.memset(oT[:], 0.0)
            vT = []
            for c in range(3):
                t = gp.tile([F, NA], f32, tag=f"vT{c}")
                nc.gpsimd.memset(t[:], 0.0)
                vT.append(t)

            # =========== geometry ===========
            V = gp.tile([NA, NA, 3], f32, tag="V")      # vec[a, b, c] = R[a,c] - R[b,c]
            Ra = W["R"][:]
            Ra_b = bass.AP(tensor=Ra.tensor, offset=Ra.offset,
                           ap=[Ra.ap[0], [0, NA], [Ra.ap[1][0], 3]])
            nc.vector.tensor_sub(V[:], Ra_b, Rb[:].rearrange("p (b c) -> p b c", c=3))
            V2 = sp.tile([NA, NA, 3], f32, tag="V2")
            nc.vector.tensor_mul(V2[:], V[:], V[:])
            d2 = sp.tile([NA, NA], f32, tag="d2")
            nc.vector.reduce_sum(d2[:], V2[:], axis=AX.X)
            lnd2 = sp.tile([NA, NA], f32, tag="lnd2")
            nc.scalar.activation(lnd2[:], d2[:], AF.Ln, bias=b30[:])
            dmat = gp.tile([NA, NA], f32, tag="dmat")   # d = exp(0.5*ln(d2))
            nc.scalar.activation(dmat[:], lnd2[:], AF.Exp, scale=0.5)
            dsafe = sp.tile([NA, NA], f32, tag="dsafe")
            nc.vector.tensor_add(dsafe[:], dmat[:], W["diagI"][:])
            invd = sp.tile([NA, NA], f32, tag="invd")
            nc.vector.reciprocal(invd[:], dsafe[:])
            vn = gp.tile([NA, NA, 3], f32, tag="vn")    # vec_norm (diag exactly 0)
            iap = invd[:]
            nc.vector.tensor_mul(vn[:], V[:], bass.AP(tensor=iap.tensor, offset=iap.offset,
                                                      ap=[iap.ap[0], [iap.ap[1][0], NA], [0, 3]]))
            vn_b = gp.tile([NA, NA, 3], bf16, tag="vn_b", name="vn_b")
            nc.vector.tensor_copy(vn_b[:], vn[:])

            # cutoff: co = 0.5*(cos(pi*d/RC)+1)*(d<=RC), diag zero
            stepm = sp.tile([NA, NA], f32, tag="stepm")
            nc.vector.tensor_scalar(out=stepm[:], in0=dmat[:], scalar1=RC, scalar2=None,
                                    op0=ALU.is_le)
            d_cl = sp.tile([NA, NA], f32, tag="d_cl")
            nc.vector.tensor_scalar(out=d_cl[:], in0=dmat[:], scalar1=RC, scalar2=None,
                                    op0=ALU.min)
            s1 = sp.tile([NA, NA], f32, tag="s1")
            nc.scalar.activation(s1[:], d_cl[:], AF.Sin, bias=bpi2[:], scale=-PI / RC)
            m32 = sp.tile([NA, NA], f32, tag="m32")
            nc.vector.tensor_mul(m32[:], stepm[:], W["halfdmask"][:])
            co_a = gp.tile([NA, NA], f32, tag="co_a")
            nc.vector.scalar_tensor_tensor(co_a[:], s1[:], 1.0, m32[:],
                                           op0=ALU.add, op1=ALU.mult)
            co_b = gp.tile([NA, NA], bf16, tag="co_b", name="co_b")
            nc.vector.tensor_copy(co_b[:], co_a[:])

            # row-ized [1, NE] copies (spread across DMA queues)
            dE = gp.tile([1, NE], f32, tag="dE", name="dE")
            nc.sync.dma_start(out=dE[:], in_=dmat[:])
            coE = gp.tile([1, NE], bf16, tag="coE", name="coE")
            nc.sync.dma_start(out=coE[:], in_=co_b[:])
            vnrow = [gp.tile([1, NE], bf16, tag=f"vnrow{c}", name=f"vnrow{c}")
                     for c in range(3)]
            nc.sync.dma_start(out=vnrow[0][:], in_=vn_b[:, :, 0])
            nc.sync.dma_start(out=vnrow[1][:], in_=vn_b[:, :, 1])
            nc.sync.dma_start(out=vnrow[2][:], in_=vn_b[:, :, 2])
            # late weight chunks: gate each behind the last rowize DMA so they
            # cannot grab the serial DMA engines before the geometry rows land
            for cn in ["W1", "W2", "W3", "WF"]:
                offs, C = chunks[cn]
                dt = f32 if cn in ("WallA", "WF") else bf16
                t = cp.tile([F, C], dt, tag=cn, name=cn)
                nc.vector.tensor_copy(t[0:1, 0:1], vnrow[2][0:1, 0:1])
                nc.sync.dma_start(out=t[:], in_=dram[cn][:])
                walls[cn] = t
                for n, (c0, p, w) in offs.items():
                    W[n] = t[0:p, c0:c0 + w]

            # RBF edge features erbf[K, NE] = exp(-gamma (d - mu)^2), bf16
            erbf = gp.tile([K, NE], bf16, tag="erbf", name="erbf")
            for hh_ in range(2):
                sl = slice(hh_ * 512, (hh_ + 1) * 512)
                pb = psE.tile([K, 512], f32, tag="edge", name="pb_rbf")
                nc.tensor.matmul(pb[:], W["ones1"][0:1, 0:K], dE[:, sl],
                                 start=True, stop=True)
                sq = sp.tile([K, 512], f32, tag="rbf_sq")
                nc.scalar.activation(sq[:], pb[:], AF.Square, bias=W["mub"][:],
                                     scale=float(np.sqrt(GAMMA)))
                nc.scalar.activation(erbf[:, sl], sq[:], AF.Exp, scale=-1.0)

            # broadcast co50 [K,NE] now (feeds e_full); co128/vnE are deferred
            # until after layer-0's edge MLP so they don't block its silus
            co50 = gp.tile([K, NE], bf16, tag="co50")
            for hh_ in range(2):
                sl = slice(hh_ * 512, (hh_ + 1) * 512)
                pb = psE.tile([K, 512], f32, tag="edge", name="pb_co50")
                nc.tensor.matmul(pb[:], W["ones1h"][0:1, 0:K], coE[:, sl],
                                 start=True, stop=True)
                nc.scalar.copy(co50[:, sl], pb[:])

            def bcast_co_vne():
                co128 = gp.tile([F, NE], bf16, tag="co128")
                for hh_ in range(2):
                    sl = slice(hh_ * 512, (hh_ + 1) * 512)
                    pb2 = psE.tile([F, 512], f32, tag="edge", name="pb_co128")
                    nc.tensor.matmul(pb2[:], W["ones1h"][:], coE[:, sl],
                                     start=True, stop=True)
                    nc.scalar.copy(co128[:, sl], pb2[:])
                vnE = []
                for c in range(3):
                    t = gp.tile([F, NE], bf16, tag=f"vnE{c}", name=f"vnE{c}")
                    for hh_ in range(2):
                        sl = slice(hh_ * 512, (hh_ + 1) * 512)
                        pb = psE.tile([F, 512], f32, tag="edge", name="pb_vne")
                        nc.tensor.matmul(pb[:], W["ones1h"][:], vnrow[c][:, sl],
                                         start=True, stop=True)
                        if c == 2:
                            nc.scalar.copy(t[:, sl], pb[:])
                        else:
                            nc.vector.tensor_copy(t[:, sl], pb[:])
                    vnE.append(t)
                return co128, vnE

            e_full = gp.tile([K, NE], bf16, tag="e_full", name="e_full")
            nc.vector.tensor_mul(e_full[:], erbf[:], co50[:])

            def e3(t):
                return t[:].rearrange("p (a b) -> p a b", a=NA)

            # edge MLP for one layer: dk + dv1/2/3 with co folded into dv
            def edge_mlp(l):
                dk = gp.tile([F, NE], bf16, tag=f"dk{l}")
                for h in range(2):
                    sl = slice(h * 512, (h + 1) * 512)
                    pm = psE.tile([F, 512], f32, tag="edge")
                    nc.tensor.matmul(pm[:], W[f"Wdk{l}"], e_full[:, sl],
                                     start=True, stop=True)
                    nc.scalar.activation(dk[:, sl], pm[:], AF.Silu, bias=W[f"bdk{l}"][:])
                dvl = []
                for c in range(3):
                    dv = gp.tile([F, NE], bf16, tag=f"dv{l}_{c}")
                    for h in range(2):
                        sl = slice(h * 512, (h + 1) * 512)
                        pm = psE.tile([F, 512], f32, tag="edge")
                        nc.tensor.matmul(pm[:], W[f"Wdv{l}"][:, c * F:(c + 1) * F],
                                         e_full[:, sl], start=True, stop=True)
                        nc.scalar.activation(dv[:, sl], pm[:], AF.Silu,
                                             bias=W[f"bdv{l}"][:, c:c + 1])
                    dvl.append(dv)
                return dk, dvl

            def layernorm_f(inT):
                # LN stats over the feature (partition) axis via PE ones-matmuls
                sq = sp.tile([F, NA], f32, tag="lnsq")
                nc.scalar.activation(sq[:], inT[:], AF.Square)
                stat = psN.tile([1, 2 * NA], f32, tag="nst")
                nc.tensor.matmul(stat[:, 0:NA], W["ones128inv"][:], inT[:],
                                 start=True, stop=True)
                nc.tensor.matmul(stat[:, NA:2 * NA], W["ones128inv"][:], sq[:],
                                 start=True, stop=True)
                statm = stat[:, 0:NA]
                varr = sp.tile([1, NA], f32, tag="varr")
                nc.scalar.activation(varr[:], statm, AF.Square)
                nc.vector.tensor_sub(varr[:], stat[:, NA:2 * NA], varr[:])
                rb = sp.tile([1, 2 * NA], f32, tag="rb")
                lnv = sp.tile([1, NA], f32, tag="lnv")
                nc.scalar.activation(lnv[:], varr[:], AF.Ln, bias=beps[:])
                nc.scalar.activation(rb[:, 0:NA], lnv[:], AF.Exp, scale=-0.5)   # rstd
                nc.vector.tensor_mul(rb[:, NA:2 * NA], statm, rb[:, 0:NA])      # mu*rstd
                bc = psN.tile([F, 2 * NA], f32, tag="nst")
                nc.tensor.matmul(bc[:], W["ones1"][:], rb[:], start=True, stop=True)
                xh = sp.tile([F, NA], f32, tag="xhatT")
                nc.vector.tensor_mul(xh[:], inT[:], bc[:, 0:NA])
                nc.vector.tensor_sub(xh[:], xh[:], bc[:, NA:2 * NA])
                return xh

            # layer-0 edge MLP emitted before the loop so ACT starts early
            dks, dvs = [None] * L, [None] * L
            dks[0], dvs[0] = edge_mlp(0)
            co128, vnE = bcast_co_vne()

            # =========== interaction layers ===========
            for l in range(L):
                first = l == 0

                # LN first: its PE stats must not queue behind the U-matmuls
                # (which wait on late vT updates from the previous layer)
                xhatT = layernorm_f(sT)
                xhb = sp.tile([F, NA], bf16, tag="xhb")
                nc.vector.tensor_copy(xhb[:], xhatT[:])

                # U-matmuls + dot chain: need only prev-layer vT; overlap LN
                ou = psS.tile([F, 512], f32, tag="qkv")
                if not first:
                    vTb = []
                    for c in range(3):
                        vb = sp.tile([F, NA], bf16, tag=f"vTb{c}")
                        nc.gpsimd.tensor_copy(vb[:], vT[c][:])
                        vTb.append(vb)
                    for c in range(3):
                        nc.tensor.matmul(ou[:, (3 + c) * NA:(4 + c) * NA], W[f"U1{l}"][:],
                                         vTb[c][:], start=True, stop=True)
                        nc.tensor.matmul(ou[:, (6 + c) * NA:(7 + c) * NA], W[f"U2{l}"][:],
                                         vTb[c][:], start=True, stop=True)
                        nc.tensor.matmul(ou[:, (9 + c) * NA:(10 + c) * NA], W[f"U3{l}"][:],
                                         vTb[c][:], start=True, stop=True)
                    us = sp.tile([F, 9 * NA], f32, tag="us")
                    nc.vector.tensor_copy(us[:], ou[:, 3 * NA:12 * NA])
                    dot = sp.tile([F, NA], f32, tag="dot")
                    pc = sp.tile([F, NA], f32, tag="dotp")
                    nc.gpsimd.tensor_mul(dot[:], us[:, 0:NA], us[:, 3 * NA:4 * NA])
                    for c in range(1, 3):
                        nc.gpsimd.tensor_mul(pc[:], us[:, c * NA:(c + 1) * NA],
                                             us[:, (3 + c) * NA:(4 + c) * NA])
                        nc.gpsimd.tensor_add(dot[:], dot[:], pc[:])

                # node matmuls with PE-accumulated biases; all stay in PSUM
                qkv = psS.tile([F, 512], f32, tag="qkv")

                def node_mm(dst, wap, brow):
                    nc.tensor.matmul(dst, brow, W["ones32h"][:], start=True, stop=False)
                    nc.tensor.matmul(dst, wap, xhb[:], start=False, stop=True)

                node_mm(qkv[:, 0:NA], W[f"Wq{l}"][:], W[f"bqr{l}"][:])
                node_mm(qkv[:, NA:2 * NA], W[f"Wk{l}"][:], W[f"bkr{l}"][:])
                for c in range(3):
                    node_mm(qkv[:, (2 + c) * NA:(3 + c) * NA],
                            W[f"Wv{l}"][:, c * F:(c + 1) * F],
                            W[f"bvr{l}"][:, c * F:(c + 1) * F])
                q_ap = qkv[:, 0:NA]
                kb = sp.tile([F, NA], bf16, tag="kb")
                nc.vector.tensor_copy(kb[:], qkv[:, NA:2 * NA])
                k_ap = kb[:]
                val = [qkv[:, (2 + c) * NA:(3 + c) * NA] for c in range(3)]
                # bf16 copies + G on Pool (off the DVE spine)
                val1b = sp.tile([F, NA], bf16, tag="val1b")
                nc.vector.tensor_copy(val1b[:], val[0])
                val3b = sp.tile([F, NA], bf16, tag="val3b")
                nc.vector.tensor_copy(val3b[:], val[2])
                if not first:
                    val2b = sp.tile([F, NA], bf16, tag="val2b")
                    nc.vector.tensor_copy(val2b[:], val[1])
                    G = []
                    for c in range(3):
                        g = sp.tile([F, NA], bf16, tag=f"G{c}")
                        nc.gpsimd.tensor_mul(g[:], val2b[:], vT[c][:])
                        G.append(g)

                # logits: qk = q (x) k; prod = qk * dk; head-sum via HH; exp
                qk = wp.tile([F, NA, NA], bf16, tag="w")
                prod = wp.tile([F, NA, NA], bf16, tag="w")
                Xp = wp.tile([F, NE], bf16, tag="w")
                lps = psX.tile([F, NE], f32, tag="lg")
                for hch in range(2):
                    asl = slice(hch * 16, (hch + 1) * 16)      # a-halves
                    csl = slice(hch * 512, (hch + 1) * 512)
                    nc.vector.tensor_mul(qk[:, asl, :], bcast_inner(q_ap[:, asl], 16, NA),
                                         bcast_outer(k_ap, 16, NA))
                    nc.vector.tensor_mul(prod[:, asl, :], qk[:, asl, :],
                                         e3(dks[l])[:, asl, :])
                    nc.tensor.matmul(lps[:, csl],
                                     W["HH"], prod[:].rearrange("p a b -> p (a b)")[:, csl],
                                     start=True, stop=True)
                    nc.scalar.activation(Xp[:, csl], lps[:, csl], AF.Exp)

                # W1 = dv1 * val1b rides in the DVE gap while PE/ACT do exp
                W1c = wp.tile([F, NA, NA], bf16, tag="w")
                nc.vector.tensor_mul(W1c[:], e3(dvs[l][0]), bcast_outer(val1b[:], NA, NA))

                # edge MLP for the next layer rides behind this layer's ACT work
                if l + 1 < L:
                    dks[l + 1], dvs[l + 1] = edge_mlp(l + 1)

                # spine: S -> invD -> Y -> P1v -> ds -> Wo -> dx -> sT
                S = sp.tile([F, NA], f32, tag="S")
                nc.vector.reduce_sum(S[:, 0:16], e3(Xp)[:, 0:16, :], axis=AX.X)
                nc.vector.reduce_sum(S[:, 16:NA], e3(Xp)[:, 16:NA, :], axis=AX.X)
                xap = Xp[:]
                diag_ap = bass.AP(tensor=xap.tensor, offset=xap.offset,
                                  ap=[xap.ap[0], [(NA + 1) * xap.ap[1][0], NA]])
                invD = sp.tile([F, NA], f32, tag="invD")
                nc.vector.tensor_sub(invD[:], S[:], diag_ap)
                nc.vector.reciprocal(invD[:], invD[:])
                Y = wp.tile([F, NA, NA], bf16, tag="w")
                P1v = wp.tile([F, NA, NA], bf16, tag="w")
                dsT = sp.tile([F, NA], f32, tag="dsT")
                for hch in range(2):
                    asl = slice(hch * 16, (hch + 1) * 16)
                    nc.vector.tensor_mul(Y[:, asl, :], e3(Xp)[:, asl, :],
                                         e3(co128)[:, asl, :])
                    nc.vector.tensor_mul(P1v[:, asl, :], Y[:, asl, :], W1c[:, asl, :])
                    nc.vector.reduce_sum(dsT[:, asl], P1v[:, asl, :], axis=AX.X)
                dsb = sp.tile([F, NA], bf16, tag="dsb")
                nc.vector.tensor_mul(dsb[:], dsT[:], invD[:])

                def node_mm2(dst, wap, brow, mov):
                    nc.tensor.matmul(dst, brow, W["ones32h"][:], start=True, stop=False)
                    nc.tensor.matmul(dst, wap, mov, start=False, stop=True)

                for c in range(3):
                    node_mm2(ou[:, c * NA:(c + 1) * NA], W[f"Wo{l}"][:, c * F:(c + 1) * F],
                             W[f"bor{l}"][:, c * F:(c + 1) * F], dsb[:])
                o1, o2, o3 = (ou[:, c * NA:(c + 1) * NA] for c in range(3))
                dx = sp.tile([F, NA], f32, tag="dx")
                if first:
                    nc.vector.tensor_copy(dx[:], o2)
                else:
                    nc.vector.tensor_mul(dx[:], o3, dot[:])
                    nc.vector.tensor_add(dx[:], dx[:], o2)
                nc.vector.tensor_add(sT[:], sT[:], dx[:])
                nc.vector.tensor_add(oT[:], oT[:], dx[:])

                # dw messages trail the spine; only next layer's U/G need them
                X3 = wp.tile([F, NA, NA], bf16, tag="w")
                nc.vector.tensor_mul(X3[:], Y[:], bcast_outer(val3b[:], NA, NA))
                P3 = wp.tile([F, NA, NA], bf16, tag="w")
                nc.vector.tensor_mul(P3[:], X3[:], e3(dvs[l][2]))
                if not first:
                    P2 = wp.tile([F, NA, NA], bf16, tag="w")
                    nc.vector.tensor_mul(P2[:], Y[:], e3(dvs[l][1]))
                dwm = sp.tile([F, 3, NA], f32, tag="dwm")
                for c in range(3):
                    tt = wp.tile([F, NA, NA], bf16, tag="w")
                    if c == 2:
                        nc.gpsimd.tensor_mul(tt[:], P3[:], e3(vnE[c]))
                    else:
                        nc.vector.tensor_mul(tt[:], P3[:], e3(vnE[c]))
                    if not first:
                        rr = wp.tile([F, NA, NA], bf16, tag="w")
                        if c == 2:
                            nc.gpsimd.tensor_mul(rr[:], P2[:], bcast_outer(G[c][:], NA, NA))
                            nc.vector.tensor_add(tt[:], tt[:], rr[:])
                        else:
                            nc.vector.tensor_mul(rr[:], P2[:], bcast_outer(G[c][:], NA, NA))
                            nc.vector.tensor_add(tt[:], tt[:], rr[:])
                    nc.vector.reduce_sum(dwm[:, c, :], tt[:], axis=AX.X)
                    nc.vector.tensor_mul(dwm[:, c, :], dwm[:, c, :], invD[:])
                dwT = [dwm[:, c, :] for c in range(3)]
                if first:
                    for c in range(3):
                        nc.gpsimd.tensor_copy(vT[c][:], dwT[c])
                else:
                    o1s = sp.tile([F, NA], f32, tag="o1s")
                    nc.vector.tensor_copy(o1s[:], o1)
                    for c in range(3):
                        t3 = sp.tile([F, NA], f32, tag="t3")
                        nc.gpsimd.tensor_mul(t3[:], o1s[:], us[:, (6 + c) * NA:(7 + c) * NA])
                        nc.gpsimd.tensor_add(vT[c][:], vT[c][:], dwT[c])
                        nc.gpsimd.tensor_add(vT[c][:], vT[c][:], t3[:])

            # =========== final LN + output MLP ===========
            xo = layernorm_f(oT)
            y_p = psS.tile([F // 2, NA], f32, tag="qkv")
            nc.tensor.matmul(y_p[:], W["b1r"][:], W["ones32"][:], start=True, stop=False)
            nc.tensor.matmul(y_p[:], W["w1p"][:], xo[:], start=False, stop=True)
            a1 = sp.tile([F // 2, NA], f32, tag="a1")
            nc.scalar.activation(a1[:], y_p[:], AF.Silu)
            asum = sp.tile([F // 2, 1], f32, tag="asum")
            nc.vector.reduce_sum(asum[:], a1[:], axis=AX.X)
            en_p = psS.tile([1, 1], f32, tag="qkv")
            nc.tensor.matmul(en_p[:], W["w2"][:], asum[:], start=True, stop=True)
            en = sp.tile([1, 1], f32, tag="en")
            nc.vector.tensor_scalar(out=en[:], in0=en_p[:], scalar1=float(NA * b2),
                                    scalar2=None, op0=ALU.add)
            nc.sync.dma_start(out=energy[:], in_=en[:])

    _split_sync_waits(nc, mybir)
    nc.finalize()
    return nc
